# revision 21
# baseline (speedup 1.0000x reference)
"""MoE (top-2 of 8 experts + dummy identity expert) on 8 NeuronCores.

Strategy (expert parallelism, per the sharding hint):
  Launch 1 (router, token-parallel): each core computes logits -> softmax
    -> top-2 gates for its 512-token shard, fully on device (fp32 so the
    top-2 selection matches the fp32 reference bit-for-bit).
  Host all-to-all "dispatch": compact token indices per expert from the
    device-computed gates, gather+transpose token activations.
  Launch 2 (expert MLP, expert-parallel, MIXED PRECISION): core e holds
    expert e's weights. Tokens are split by routing risk s=||gates||_2:
    the top KB tokens per expert run in bf16; the low-gate remainder runs
    in fp8(e4m3) with DoubleRow perf mode (2x PE throughput). Quantization
    error scales with the gate, so low-gate tokens tolerate fp8.
  Host "combine": scatter-add per-expert outputs + dummy-expert term.
"""

import math
import os
import sys

for _p in ("/opt/trn_rl_repo",):
    if _p not in sys.path:
        sys.path.insert(0, _p)

import numpy as np
import ml_dtypes

import concourse.bass as bass
import concourse.mybir as mybir
import concourse.tile as tile
from concourse.bass import ts
from concourse.bass_utils import run_bass_kernel_spmd

# ---------------------------------------------------------------------------
# This container's walrus accepts at most ONE sync-wait command per
# instruction. Tile can attach several (body instructions and the
# kernel-tail drain). Hoist excess waits onto same-engine NoOps inserted
# immediately before the offending instruction — semantically identical
# (waits are AND conditions evaluated in stream order).
# ---------------------------------------------------------------------------
_WAITS_PER_INST = 1
_legalize_counter = [0]


def _legalize_waits(nc):
    for f in nc.m.functions:
        for bb in f.blocks:
            insts = list(bb.instructions)
            out = []
            changed = False
            for inst in insts:
                si = inst.sync_info
                waits = list(si.on_wait) if si is not None and si.on_wait else []
                if len(waits) > _WAITS_PER_INST:
                    changed = True
                    for w in waits[:-_WAITS_PER_INST]:
                        _legalize_counter[0] += 1
                        out.append(
                            mybir.InstNoOp(
                                name=f"legwait-{_legalize_counter[0]}",
                                ins=[],
                                outs=[],
                                engine=inst.engine,
                                sync_info=mybir.SyncInfo(
                                    on_wait=[w], on_update=[]
                                ),
                            )
                        )
                    si.on_wait = waits[-_WAITS_PER_INST:]
                out.append(inst)
            if changed:
                bb.instructions = out
    return nc


# ---------------------------------------------------------------------------
# Problem constants (hardcoded per contract; inputs are fixed-shape).
# ---------------------------------------------------------------------------
N_CORES = 8
B, T, D, F, E = 2, 2048, 1024, 4096, 8
NTOK = B * T            # 4096 tokens
TPC = NTOK // N_CORES   # 512 tokens/core in the router launch
P = 128
KD = D // P             # 8 contraction chunks over D
KF = F // P             # 32 contraction chunks over F

F32 = mybir.dt.float32
BF16 = mybir.dt.bfloat16
FP8 = mybir.dt.float8e4
NP_BF16 = ml_dtypes.bfloat16
NP_FP8 = ml_dtypes.float8_e4m3

# fp8 scaling: pre-psum = (SX*x) @ (SW*w1) = SX*SW*pre; gelu scale undoes it.
SX = 8.0
SW = 64.0

PROFILE = False          # set True (from test.py) to collect NTFF exec times
LAST_EXEC_NS = {}        # launch name -> exec_time_ns (filled when PROFILE)
LAST_TRACE_DIRS = {}


# ---------------------------------------------------------------------------
# Launch 1: router. Per core: 512 tokens -> gates [512, 9].
# ---------------------------------------------------------------------------
def build_router():
    NE = E + 1
    TT = TPC // P  # 4 token tiles of 128
    nc = bass.Bass()
    # x and router weights arrive as bf16 hi/lo pairs; computing
    #   x_hi@rw_hi + x_lo@rw_hi + x_hi@rw_lo
    # in fp32 PSUM reproduces fp32 logits to ~1e-5 (verified: every token's
    # error is <= 10% of its top-2 decision gap, so selection is exact)
    # while running the PE at 1 cycle/row instead of fp32's 4.
    xh = nc.declare_dram_parameter("xh", [KD, P, TPC], BF16, isOutput=False)
    xl = nc.declare_dram_parameter("xl", [KD, P, TPC], BF16, isOutput=False)
    rw2 = nc.declare_dram_parameter("rw2", [P, KD, 2, NE], BF16, isOutput=False)
    rbT = nc.declare_dram_parameter("rbT", [NE, 1], F32, isOutput=False)
    id9 = nc.declare_dram_parameter("id9", [NE, NE], F32, isOutput=False)
    gates = nc.declare_dram_parameter("gates", [TPC, NE], F32, isOutput=True)

    with tile.TileContext(nc) as tc:
        with (
            tc.tile_pool(name="const", bufs=1) as cpool,
            tc.tile_pool(name="xp", bufs=8) as xpool,
            tc.tile_pool(name="work", bufs=2) as pool,
            tc.tile_pool(name="psum", bufs=1, space="PSUM") as pp,
            tc.tile_pool(name="psum2", bufs=1, space="PSUM") as pp2,
        ):
            rw_sb = cpool.tile([P, KD, 2, NE], BF16)
            nc.sync.dma_start(rw_sb[:], rw2[:])
            rbT_sb = cpool.tile([NE, 1], F32)
            id9_sb = cpool.tile([NE, NE], F32)

            ps_lgT = pp2.tile([NE, TPC], F32)
            # The PE runs at a reduced p-state until it accumulates a
            # sustained stretch of busy time; the logits stream is DMA-paced
            # with gaps that keep resetting the ramp. Fill those gaps with
            # dummy matmuls on the chunk that just arrived (dead PSUM out)
            # so the clock is at full speed once the DMA backlog clears.
            ps_warm = pp.tile([NE, TPC], F32, name="ps_warm")
            dma_engines = [nc.sync, nc.gpsimd, nc.scalar]
            for k in range(KD):
                xh_sb = xpool.tile([P, TPC], BF16, tag="xh")
                dma_engines[(2 * k) % 3].dma_start(xh_sb[:], xh[k])
                xl_sb = xpool.tile([P, TPC], BF16, tag="xl")
                dma_engines[(2 * k + 1) % 3].dma_start(xl_sb[:], xl[k])
                if k == 1:
                    nc.scalar.dma_start(rbT_sb[:], rbT[:])
                    nc.scalar.dma_start(id9_sb[:], id9[:])
                nc.tensor.matmul(
                    ps_lgT[:], lhsT=rw_sb[:, k, 0, :], rhs=xh_sb[:],
                    start=(k == 0), stop=False,
                )
                nc.tensor.matmul(
                    ps_lgT[:], lhsT=rw_sb[:, k, 0, :], rhs=xl_sb[:],
                    start=False, stop=False,
                )
                nc.tensor.matmul(
                    ps_lgT[:], lhsT=rw_sb[:, k, 1, :], rhs=xh_sb[:],
                    start=False, stop=(k == KD - 1),
                )
                if k < 5:
                    for _ in range(2):
                        nc.tensor.matmul(
                            ps_warm[:], lhsT=rw_sb[:, k, 1, :], rhs=xh_sb[:],
                            start=True, stop=True,
                        )
            lgT_sb = pool.tile([NE, TPC], F32, tag="lgT")
            nc.vector.tensor_tensor(
                lgT_sb[:], ps_lgT[:],
                rbT_sb[:].to_broadcast([NE, TPC]),
                mybir.AluOpType.add,
            )
            # transpose back to token-major [128, 4, 9] via PE
            ps = pp.tile([P, TT, NE], F32)
            for tt in range(TT):
                nc.tensor.transpose(
                    ps[:, tt, :], lgT_sb[:, ts(tt, P)], id9_sb[:]
                )

            # logits are O(+-5): exp() cannot overflow fp32, and the
            # softmax normalization cancels any shift, so skip the max-
            # subtraction entirely (selection is order-preserving).
            sh3 = [P, TT, NE]
            e_sb = pool.tile(sh3, F32, tag="e")
            nc.scalar.activation(
                e_sb[:], ps[:], mybir.ActivationFunctionType.Exp,
            )
            s = pool.tile([P, TT], F32, tag="s")
            nc.vector.tensor_reduce(
                s[:], e_sb[:], axis=mybir.AxisListType.X, op=mybir.AluOpType.add,
            )
            mx = pool.tile([P, TT], F32, tag="mx")
            nc.vector.tensor_reduce(
                mx[:], e_sb[:], axis=mybir.AxisListType.X, op=mybir.AluOpType.max,
            )
            # knock out the top-1, take max again -> second-largest
            mlt = pool.tile(sh3, F32, tag="mlt")
            nc.vector.tensor_tensor(
                mlt[:], e_sb[:], mx[:, :, None].to_broadcast(sh3),
                mybir.AluOpType.is_lt,
            )
            emask = pool.tile(sh3, F32, tag="emask")
            nc.vector.tensor_mul(out=emask[:], in0=mlt[:], in1=e_sb[:])
            m2 = pool.tile([P, TT], F32, tag="m2")
            nc.vector.tensor_reduce(
                m2[:], emask[:], axis=mybir.AxisListType.X, op=mybir.AluOpType.max,
            )
            gmask = pool.tile(sh3, F32, tag="gmask")
            nc.vector.tensor_tensor(
                gmask[:], e_sb[:], m2[:, :, None].to_broadcast(sh3),
                mybir.AluOpType.is_ge,
            )
            gsel = pool.tile(sh3, F32, tag="gsel")
            nc.vector.tensor_mul(out=gsel[:], in0=gmask[:], in1=e_sb[:])
            rs = pool.tile([P, TT], F32, tag="rs")
            nc.vector.reciprocal(rs[:], s[:])
            gfin = pool.tile(sh3, F32, tag="gfin")
            nc.vector.tensor_tensor(
                gfin[:], gsel[:], rs[:, :, None].to_broadcast(sh3),
                mybir.AluOpType.mult,
            )
            nc.sync.dma_start(
                gates.rearrange("(tt p) e -> p tt e", p=P), gfin[:]
            )
    return _legalize_waits(nc)


# ---------------------------------------------------------------------------
# Launch 2: mixed-precision expert MLP. Per core: KB bf16 tokens + KF8 fp8
# tokens through one expert.
#   yT[d, c] = gate[c] * (gelu(x @ w1) @ w2)[c, d]   (transposed output)
# fp8 tiles use DoubleRow perf mode: lhsT [128,2,128] x rhs [128,2,512]
# contracts 256 elements per instruction at 2x bf16 throughput.
# ---------------------------------------------------------------------------
def build_mlp_mixed(KB, KF8):
    assert KB % 512 == 0 and KF8 % 512 == 0
    NB = KB // 512
    NF = KF8 // 512
    C = KB + KF8
    DR = mybir.MatmulPerfMode.DoubleRow
    nc = bass.Bass()
    w1b = nc.declare_dram_parameter("w1b", [KF // 2, P, 2, KD, P], BF16,
                                    isOutput=False)
    w1f = nc.declare_dram_parameter("w1f", [KF // 2, P, 2, KD // 2, 2, P], FP8,
                                    isOutput=False)
    w2b = nc.declare_dram_parameter("w2b", [KD, P, KF, P], BF16, isOutput=False)
    w2f = nc.declare_dram_parameter("w2f", [KD, P, KF // 2, 2, P], FP8,
                                    isOutput=False)
    xb = nc.declare_dram_parameter("xb", [P, KD, KB], BF16, isOutput=False)
    xf = nc.declare_dram_parameter("xf", [P, KD, KF8], FP8, isOutput=False)
    grep = nc.declare_dram_parameter("grep", [P, C], F32, isOutput=False)
    yT = nc.declare_dram_parameter("yT", [D, C], F32, isOutput=True)

    with tile.TileContext(nc) as tc:
        with (
            tc.tile_pool(name="const", bufs=1) as cpool,
            tc.tile_pool(name="w1bp", bufs=3) as w1bpool,
            tc.tile_pool(name="w1fp", bufs=3) as w1fpool,
            tc.tile_pool(name="w2bp", bufs=2) as w2bpool,
            tc.tile_pool(name="w2fp", bufs=2) as w2fpool,
            tc.tile_pool(name="yp", bufs=4) as ypool,
            tc.tile_pool(name="ph_b", bufs=2, space="PSUM") as phb,
            tc.tile_pool(name="ph_f", bufs=2, space="PSUM") as phf,
            tc.tile_pool(name="py_b", bufs=2, space="PSUM") as pyb,
            tc.tile_pool(name="py_f", bufs=2, space="PSUM") as pyf,
        ):
            # token activations: split the bf16 x DMA over two queues so the
            # first phase-1 group can start ASAP.
            xf_sb = cpool.tile([P, KD, KF8], FP8)
            nc.scalar.dma_start(xf_sb[:, 0:KD // 2, :], xf[:, 0:KD // 2, :])
            nc.gpsimd.dma_start(xf_sb[:, KD // 2:KD, :], xf[:, KD // 2:KD, :])
            w1f_t0 = w1fpool.tile([P, 2, KD // 2, 2, P], FP8, tag="w1f")
            nc.sync.dma_start(w1f_t0[:], w1f[0])
            xb_sb = cpool.tile([P, KD, KB], BF16)
            nc.sync.dma_start(xb_sb[:, 0:KD // 2, :], xb[:, 0:KD // 2, :])
            nc.scalar.dma_start(xb_sb[:, KD // 2:KD, :], xb[:, KD // 2:KD, :])
            grep_sb = cpool.tile([P, C], F32)

            hb_sb = cpool.tile([P, KF, KB], BF16)
            hf_sb = cpool.tile([P, KF, KF8], FP8)

            # phase 1: h = gelu(x @ w1); bf16 tile then fp8 tile per
            # f-chunk; w1 streams in two-f-chunk transfers to halve the
            # completion-event count.
            for f in range(KF):
                if f % 2 == 0:
                    w1b_t = w1bpool.tile([P, 2, KD, P], BF16, tag="w1b")
                    nc.gpsimd.dma_start(w1b_t[:], w1b[f // 2])
                    if f == 0:
                        w1f_t = w1f_t0
                    else:
                        w1f_t = w1fpool.tile([P, 2, KD // 2, 2, P], FP8,
                                             tag="w1f")
                        nc.sync.dma_start(w1f_t[:], w1f[f // 2])
                if f == 6:
                    nc.scalar.dma_start(grep_sb[:], grep[:])
                fi = f % 2
                for t in range(NF):
                    ps = phf.tile([P, 512], F32)
                    for kk in range(KD // 2):
                        nc.tensor.matmul(
                            ps[:],
                            lhsT=w1f_t[:, fi, kk, :, :],
                            rhs=xf_sb[:, 2 * kk:2 * kk + 2, ts(t, 512)],
                            start=(kk == 0),
                            stop=(kk == KD // 2 - 1),
                            perf_mode=DR,
                        )
                    nc.scalar.activation(
                        hf_sb[:, f, ts(t, 512)], ps[:],
                        mybir.ActivationFunctionType.Gelu,
                        scale=1.0 / (SX * SW),
                    )
                for t in range(NB):
                    ps = phb.tile([P, 512], F32)
                    if f == 0:
                        # ramp the PE clock while the larger xb/w1b inputs
                        # stream in: back-to-back dummy matmuls on the
                        # already-resident fp8 tiles keep the PE busy through
                        # the DMA-bound warmup window. They write this psum
                        # tile, which the real k==0 matmul resets (start=True).
                        for _ in range(10):
                            nc.tensor.matmul(
                                ps[:], lhsT=w1f_t0[:, 0, 0, 0, :],
                                rhs=xf_sb[:, 0, :],
                                start=True, stop=True,
                            )
                    for k in range(KD):
                        nc.tensor.matmul(
                            ps[:],
                            lhsT=w1b_t[:, fi, k, :],
                            rhs=xb_sb[:, k, ts(t, 512)],
                            start=(k == 0),
                            stop=(k == KD - 1),
                        )
                    nc.scalar.activation(
                        hb_sb[:, f, ts(t, 512)], ps[:],
                        mybir.ActivationFunctionType.Gelu,
                    )

            # phase 2: yT[d, c] = gate[c] * sum_f w2[f, d] * h[f, c]
            for d in range(KD):
                w2b_t = w2bpool.tile([P, KF, P], BF16, tag="w2b")
                nc.gpsimd.dma_start(w2b_t[:], w2b[d])
                w2f_t = w2fpool.tile([P, KF // 2, 2, P], FP8, tag="w2f")
                nc.gpsimd.dma_start(w2f_t[:], w2f[d])
                for t in range(NB):
                    ps = pyb.tile([P, 512], F32)
                    for k in range(KF):
                        nc.tensor.matmul(
                            ps[:],
                            lhsT=w2b_t[:, k, :],
                            rhs=hb_sb[:, k, ts(t, 512)],
                            start=(k == 0),
                            stop=(k == KF - 1),
                        )
                    y_sb = ypool.tile([P, 512], F32, tag="yb")
                    nc.vector.tensor_mul(
                        out=y_sb[:], in0=ps[:], in1=grep_sb[:, ts(t, 512)]
                    )
                    nc.sync.dma_start(yT[ts(d, P), ts(t, 512)], y_sb[:])
                for t in range(NF):
                    ps = pyf.tile([P, 512], F32)
                    for kk in range(KF // 2):
                        nc.tensor.matmul(
                            ps[:],
                            lhsT=w2f_t[:, kk, :, :],
                            rhs=hf_sb[:, 2 * kk:2 * kk + 2, ts(t, 512)],
                            start=(kk == 0),
                            stop=(kk == KF // 2 - 1),
                            perf_mode=DR,
                        )
                    y_sb = ypool.tile([P, 512], F32, tag="yf")
                    nc.vector.tensor_mul(
                        out=y_sb[:], in0=ps[:],
                        in1=grep_sb[:, KB + t * 512:KB + (t + 1) * 512],
                    )
                    nc.sync.dma_start(
                        yT[ts(d, P), KB + t * 512:KB + (t + 1) * 512], y_sb[:]
                    )
    return _legalize_waits(nc)


_BUILT = {}


def _get_router():
    if "router" not in _BUILT:
        _BUILT["router"] = build_router()
    return _BUILT["router"]


def _get_mlp(KB, KF8):
    key = ("mlp", KB, KF8)
    if key not in _BUILT:
        _BUILT[key] = build_mlp_mixed(KB, KF8)
    return _BUILT[key]


def _run(name, nc, in_maps):
    kw = {}
    if PROFILE:
        kw["trace"] = True
    res = run_bass_kernel_spmd(nc, in_maps, core_ids=list(range(N_CORES)), **kw)
    if PROFILE:
        LAST_EXEC_NS[name] = res.exec_time_ns
        LAST_TRACE_DIRS[name] = getattr(res, "profile_json", None)
    return res.results


# ---------------------------------------------------------------------------
# host-side packing helpers
# ---------------------------------------------------------------------------
def _part3(a, np_dt):
    """[K*P, N] -> [P, K, N] with partition dim first (contiguous)."""
    kp, n = a.shape
    k = kp // P
    return np.ascontiguousarray(
        a.reshape(k, P, n).transpose(1, 0, 2).astype(np_dt, copy=False)
    )


def _xT_pack(xg, np_dt):
    """[C, D] tokens -> [P, KD, C] (d-major, partition-first)."""
    return _part3(np.ascontiguousarray(xg.T), np_dt)


def kernel(x, router_w, router_b, w1, w2):
    x = np.asarray(x, dtype=np.float32)
    router_w = np.asarray(router_w, dtype=np.float32)
    router_b = np.asarray(router_b, dtype=np.float32)
    w1 = np.asarray(w1, dtype=np.float32)
    w2 = np.asarray(w2, dtype=np.float32)

    xf_ = x.reshape(NTOK, D)

    # ---- launch 1: router -------------------------------------------------
    rw_f = _part3(router_w, np.float32)                    # [128, 8, 9]
    rw_hi = rw_f.astype(NP_BF16)
    rw_lo = (rw_f - rw_hi.astype(np.float32)).astype(NP_BF16)
    rw2_h = np.ascontiguousarray(
        np.stack([rw_hi, rw_lo], axis=2))                  # [128, 8, 2, 9]
    rbT_h = np.ascontiguousarray(router_b.reshape(E + 1, 1))
    id9_h = np.eye(E + 1, dtype=np.float32)
    in_maps = []
    for c in range(N_CORES):
        xs = xf_[c * TPC:(c + 1) * TPC]                    # [512, 1024]
        xT_h = np.ascontiguousarray(xs.T).reshape(KD, P, TPC)  # [8, 128, 512]
        xh_h = xT_h.astype(NP_BF16)
        xl_h = (xT_h - xh_h.astype(np.float32)).astype(NP_BF16)
        in_maps.append({"xh": xh_h, "xl": xl_h, "rw2": rw2_h,
                        "rbT": rbT_h, "id9": id9_h})
    results = _run("router", _get_router(), in_maps)
    gates = np.concatenate(
        [np.asarray(r["gates"], dtype=np.float32) for r in results], axis=0
    )                                                      # [4096, 9]

    # ---- host all-to-all dispatch + precision split ----------------------
    idx = [np.nonzero(gates[:, e] > 0)[0] for e in range(E)]
    KB = 512
    # per-expert: the KB highest-gate assignments run in bf16; the rest fp8.
    b_idx, f_idx = [], []
    for e in range(E):
        ide = idx[e]
        r = np.argsort(-gates[ide, e], kind="stable")
        b_idx.append(ide[r[:KB]])
        f_idx.append(ide[r[KB:]])
    max_f = max((len(i) for i in f_idx), default=0)
    KF8 = max(512, ((max_f + 511) // 512) * 512)

    nc_mlp = _get_mlp(KB, KF8)
    in_maps = []
    for e in range(E):
        w1_bl = np.stack(
            [
                w1[e][:, f * P:(f + 1) * P].reshape(KD, P, P).transpose(1, 0, 2)
                for f in range(KF)
            ]
        )                                                  # [32, 128, 8, 128] f32
        w2_bl = np.stack(
            [
                w2[e][:, d * P:(d + 1) * P].reshape(KF, P, P).transpose(1, 0, 2)
                for d in range(KD)
            ]
        )                                                  # [8, 128, 32, 128] f32
        w1b_h = w1_bl.astype(NP_BF16).reshape(KF // 2, 2, P, KD, P) \
            .transpose(0, 2, 1, 3, 4)
        w2b_h = w2_bl.astype(NP_BF16)
        w1f_h = (w1_bl * SW).astype(NP_FP8) \
            .reshape(KF // 2, 2, P, KD // 2, 2, P).transpose(0, 2, 1, 3, 4, 5)
        w2f_h = (w2_bl * SW).astype(NP_FP8).reshape(KD, P, KF // 2, 2, P)

        ib, if8 = b_idx[e], f_idx[e]
        nb, nf = len(ib), len(if8)
        xg_b = np.zeros((KB, D), dtype=np.float32)
        xg_b[:nb] = xf_[ib]
        xg_f = np.zeros((KF8, D), dtype=np.float32)
        xg_f[:nf] = xf_[if8]
        g = np.zeros((KB + KF8,), dtype=np.float32)
        g[:nb] = gates[ib, e]
        g[KB:KB + nf] = gates[if8, e] / (SW)

        in_maps.append({
            "w1b": np.ascontiguousarray(w1b_h),
            "w1f": np.ascontiguousarray(w1f_h),
            "w2b": np.ascontiguousarray(w2b_h),
            "w2f": np.ascontiguousarray(w2f_h),
            "xb": _xT_pack(xg_b, NP_BF16),
            "xf": _xT_pack(xg_f * SX, NP_FP8),
            "grep": np.ascontiguousarray(np.broadcast_to(g, (P, KB + KF8))),
        })

    # ---- launch 2: expert MLP --------------------------------------------
    results = _run("mlp", nc_mlp, in_maps)

    # ---- host combine -----------------------------------------------------
    out = gates[:, E:E + 1] * xf_                          # dummy identity expert
    for e in range(E):
        yT = np.asarray(results[e]["yT"], dtype=np.float32)    # [1024, KB+KF8]
        nb, nf = len(b_idx[e]), len(f_idx[e])
        if nb:
            out[b_idx[e]] += yT[:, :nb].T
        if nf:
            out[f_idx[e]] += yT[:, KB:KB + nf].T
    return out.reshape(B, T, D).astype(np.float32)


# revision 22
# speedup vs baseline: 1.1781x; 1.1781x over previous
"""MoE (top-2 of 8 experts + dummy identity expert) on 8 NeuronCores.

Strategy (expert parallelism, per the sharding hint):
  Launch 1 (router, token-parallel): each core computes logits -> softmax
    -> top-2 gates for its 512-token shard, fully on device (fp32 so the
    top-2 selection matches the fp32 reference bit-for-bit).
  Host all-to-all "dispatch": compact token indices per expert from the
    device-computed gates, gather+transpose token activations.
  Launch 2 (expert MLP, expert-parallel, MIXED PRECISION): core e holds
    expert e's weights. Tokens are split by routing risk s=||gates||_2:
    the top KB tokens per expert run in bf16; the low-gate remainder runs
    in fp8(e4m3) with DoubleRow perf mode (2x PE throughput). Quantization
    error scales with the gate, so low-gate tokens tolerate fp8.
  Host "combine": scatter-add per-expert outputs + dummy-expert term.
"""

import math
import os
import sys

for _p in ("/opt/trn_rl_repo",):
    if _p not in sys.path:
        sys.path.insert(0, _p)

import numpy as np
import ml_dtypes

import concourse.bass as bass
import concourse.mybir as mybir
import concourse.tile as tile
from concourse.bass import ts
from concourse.bass_utils import run_bass_kernel_spmd

# ---------------------------------------------------------------------------
# This container's walrus accepts at most ONE sync-wait command per
# instruction. Tile can attach several (body instructions and the
# kernel-tail drain). Hoist excess waits onto same-engine NoOps inserted
# immediately before the offending instruction — semantically identical
# (waits are AND conditions evaluated in stream order).
# ---------------------------------------------------------------------------
_WAITS_PER_INST = 1
_legalize_counter = [0]


def _legalize_waits(nc):
    for f in nc.m.functions:
        for bb in f.blocks:
            insts = list(bb.instructions)
            out = []
            changed = False
            for inst in insts:
                si = inst.sync_info
                waits = list(si.on_wait) if si is not None and si.on_wait else []
                if len(waits) > _WAITS_PER_INST:
                    changed = True
                    for w in waits[:-_WAITS_PER_INST]:
                        _legalize_counter[0] += 1
                        out.append(
                            mybir.InstNoOp(
                                name=f"legwait-{_legalize_counter[0]}",
                                ins=[],
                                outs=[],
                                engine=inst.engine,
                                sync_info=mybir.SyncInfo(
                                    on_wait=[w], on_update=[]
                                ),
                            )
                        )
                    si.on_wait = waits[-_WAITS_PER_INST:]
                out.append(inst)
            if changed:
                bb.instructions = out
    return nc


# ---------------------------------------------------------------------------
# Problem constants (hardcoded per contract; inputs are fixed-shape).
# ---------------------------------------------------------------------------
N_CORES = 8
B, T, D, F, E = 2, 2048, 1024, 4096, 8
NTOK = B * T            # 4096 tokens
TPC = NTOK // N_CORES   # 512 tokens/core in the router launch
P = 128
KD = D // P             # 8 contraction chunks over D
KF = F // P             # 32 contraction chunks over F

F32 = mybir.dt.float32
BF16 = mybir.dt.bfloat16
FP8 = mybir.dt.float8e4
NP_BF16 = ml_dtypes.bfloat16
NP_FP8 = ml_dtypes.float8_e4m3

# fp8 scaling: pre-psum = (SX*x) @ (SW*w1) = SX*SW*pre; gelu scale undoes it.
SX = 8.0
SW = 64.0

PROFILE = False          # set True (from test.py) to collect NTFF exec times
LAST_EXEC_NS = {}        # launch name -> exec_time_ns (filled when PROFILE)
LAST_TRACE_DIRS = {}


# ---------------------------------------------------------------------------
# Launch 1: router. Per core: 512 tokens -> gates [512, 9].
# ---------------------------------------------------------------------------
def build_router():
    NE = E + 1
    TT = TPC // P  # 4 token tiles of 128
    nc = bass.Bass()
    # x and router weights arrive as bf16 hi/lo pairs; computing
    #   x_hi@rw_hi + x_lo@rw_hi + x_hi@rw_lo
    # in fp32 PSUM reproduces fp32 logits to ~1e-5 (verified: every token's
    # error is <= 10% of its top-2 decision gap, so selection is exact)
    # while running the PE at 1 cycle/row instead of fp32's 4.
    xh = nc.declare_dram_parameter("xh", [KD, P, TPC], BF16, isOutput=False)
    xl = nc.declare_dram_parameter("xl", [KD, P, TPC], BF16, isOutput=False)
    rw2 = nc.declare_dram_parameter("rw2", [P, KD, 2, NE], BF16, isOutput=False)
    rbT = nc.declare_dram_parameter("rbT", [NE, 1], F32, isOutput=False)
    id9 = nc.declare_dram_parameter("id9", [NE, NE], F32, isOutput=False)
    gates = nc.declare_dram_parameter("gates", [TPC, NE], F32, isOutput=True)

    with tile.TileContext(nc) as tc:
        with (
            tc.tile_pool(name="const", bufs=1) as cpool,
            tc.tile_pool(name="xp", bufs=8) as xpool,
            tc.tile_pool(name="work", bufs=2) as pool,
            tc.tile_pool(name="psum", bufs=1, space="PSUM") as pp,
            tc.tile_pool(name="psum2", bufs=1, space="PSUM") as pp2,
        ):
            rw_sb = cpool.tile([P, KD, 2, NE], BF16)
            nc.sync.dma_start(rw_sb[:], rw2[:])
            rbT_sb = cpool.tile([NE, 1], F32)
            id9_sb = cpool.tile([NE, NE], F32)

            ps_lgT = pp2.tile([NE, TPC], F32)
            dma_engines = [nc.sync, nc.gpsimd, nc.scalar]
            for k in range(KD):
                xh_sb = xpool.tile([P, TPC], BF16, tag="xh")
                dma_engines[(2 * k) % 3].dma_start(xh_sb[:], xh[k])
                xl_sb = xpool.tile([P, TPC], BF16, tag="xl")
                dma_engines[(2 * k + 1) % 3].dma_start(xl_sb[:], xl[k])
                if k == 1:
                    nc.scalar.dma_start(rbT_sb[:], rbT[:])
                    nc.scalar.dma_start(id9_sb[:], id9[:])
                nc.tensor.matmul(
                    ps_lgT[:], lhsT=rw_sb[:, k, 0, :], rhs=xh_sb[:],
                    start=(k == 0), stop=False,
                )
                nc.tensor.matmul(
                    ps_lgT[:], lhsT=rw_sb[:, k, 0, :], rhs=xl_sb[:],
                    start=False, stop=False,
                )
                nc.tensor.matmul(
                    ps_lgT[:], lhsT=rw_sb[:, k, 1, :], rhs=xh_sb[:],
                    start=False, stop=(k == KD - 1),
                )
            lgT_sb = pool.tile([NE, TPC], F32, tag="lgT")
            nc.vector.tensor_tensor(
                lgT_sb[:], ps_lgT[:],
                rbT_sb[:].to_broadcast([NE, TPC]),
                mybir.AluOpType.add,
            )
            # transpose back to token-major [128, 4, 9] via PE
            ps = pp.tile([P, TT, NE], F32)
            for tt in range(TT):
                nc.tensor.transpose(
                    ps[:, tt, :], lgT_sb[:, ts(tt, P)], id9_sb[:]
                )

            # logits are O(+-5): exp() cannot overflow fp32, and the
            # softmax normalization cancels any shift, so skip the max-
            # subtraction entirely (selection is order-preserving).
            sh3 = [P, TT, NE]
            e_sb = pool.tile(sh3, F32, tag="e")
            nc.scalar.activation(
                e_sb[:], ps[:], mybir.ActivationFunctionType.Exp,
            )
            s = pool.tile([P, TT], F32, tag="s")
            nc.vector.tensor_reduce(
                s[:], e_sb[:], axis=mybir.AxisListType.X, op=mybir.AluOpType.add,
            )
            mx = pool.tile([P, TT], F32, tag="mx")
            nc.vector.tensor_reduce(
                mx[:], e_sb[:], axis=mybir.AxisListType.X, op=mybir.AluOpType.max,
            )
            # knock out the top-1, take max again -> second-largest
            mlt = pool.tile(sh3, F32, tag="mlt")
            nc.vector.tensor_tensor(
                mlt[:], e_sb[:], mx[:, :, None].to_broadcast(sh3),
                mybir.AluOpType.is_lt,
            )
            emask = pool.tile(sh3, F32, tag="emask")
            nc.vector.tensor_mul(out=emask[:], in0=mlt[:], in1=e_sb[:])
            m2 = pool.tile([P, TT], F32, tag="m2")
            nc.vector.tensor_reduce(
                m2[:], emask[:], axis=mybir.AxisListType.X, op=mybir.AluOpType.max,
            )
            gmask = pool.tile(sh3, F32, tag="gmask")
            nc.vector.tensor_tensor(
                gmask[:], e_sb[:], m2[:, :, None].to_broadcast(sh3),
                mybir.AluOpType.is_ge,
            )
            gsel = pool.tile(sh3, F32, tag="gsel")
            nc.vector.tensor_mul(out=gsel[:], in0=gmask[:], in1=e_sb[:])
            rs = pool.tile([P, TT], F32, tag="rs")
            nc.vector.reciprocal(rs[:], s[:])
            gfin = pool.tile(sh3, F32, tag="gfin")
            nc.vector.tensor_tensor(
                gfin[:], gsel[:], rs[:, :, None].to_broadcast(sh3),
                mybir.AluOpType.mult,
            )
            nc.sync.dma_start(
                gates.rearrange("(tt p) e -> p tt e", p=P), gfin[:]
            )
    return _legalize_waits(nc)


# ---------------------------------------------------------------------------
# Launch 2: mixed-precision expert MLP. Per core: KB bf16 tokens + KF8 fp8
# tokens through one expert.
#   yT[d, c] = gate[c] * (gelu(x @ w1) @ w2)[c, d]   (transposed output)
# fp8 tiles use DoubleRow perf mode: lhsT [128,2,128] x rhs [128,2,512]
# contracts 256 elements per instruction at 2x bf16 throughput.
# ---------------------------------------------------------------------------
def build_mlp_mixed(KB, KF8):
    assert KB % 512 == 0 and KF8 % 512 == 0
    NB = KB // 512
    NF = KF8 // 512
    C = KB + KF8
    DR = mybir.MatmulPerfMode.DoubleRow
    nc = bass.Bass()
    w1b = nc.declare_dram_parameter("w1b", [KF // 2, P, 2, KD, P], BF16,
                                    isOutput=False)
    w1f = nc.declare_dram_parameter("w1f", [KF // 2, P, 2, KD // 2, 2, P], FP8,
                                    isOutput=False)
    w2b = nc.declare_dram_parameter("w2b", [KD, P, KF, P], BF16, isOutput=False)
    w2f = nc.declare_dram_parameter("w2f", [KD, P, KF // 2, 2, P], FP8,
                                    isOutput=False)
    xb = nc.declare_dram_parameter("xb", [P, KD, KB], BF16, isOutput=False)
    xf = nc.declare_dram_parameter("xf", [P, KD, KF8], FP8, isOutput=False)
    grep = nc.declare_dram_parameter("grep", [P, C], F32, isOutput=False)
    yT = nc.declare_dram_parameter("yT", [D, C], F32, isOutput=True)

    with tile.TileContext(nc) as tc:
        with (
            tc.tile_pool(name="const", bufs=1) as cpool,
            tc.tile_pool(name="w1bp", bufs=3) as w1bpool,
            tc.tile_pool(name="w1fp", bufs=3) as w1fpool,
            tc.tile_pool(name="w2bp", bufs=2) as w2bpool,
            tc.tile_pool(name="w2fp", bufs=2) as w2fpool,
            tc.tile_pool(name="yp", bufs=4) as ypool,
            tc.tile_pool(name="ph_b", bufs=2, space="PSUM") as phb,
            tc.tile_pool(name="ph_f", bufs=2, space="PSUM") as phf,
            tc.tile_pool(name="py_b", bufs=2, space="PSUM") as pyb,
            tc.tile_pool(name="py_f", bufs=2, space="PSUM") as pyf,
        ):
            # token activations: split the bf16 x DMA over two queues so the
            # first phase-1 group can start ASAP.
            xf_sb = cpool.tile([P, KD, KF8], FP8)
            nc.scalar.dma_start(xf_sb[:, 0:KD // 2, :], xf[:, 0:KD // 2, :])
            nc.gpsimd.dma_start(xf_sb[:, KD // 2:KD, :], xf[:, KD // 2:KD, :])
            w1f_t0 = w1fpool.tile([P, 2, KD // 2, 2, P], FP8, tag="w1f")
            nc.sync.dma_start(w1f_t0[:], w1f[0])
            xb_sb = cpool.tile([P, KD, KB], BF16)
            nc.sync.dma_start(xb_sb[:, 0:KD // 2, :], xb[:, 0:KD // 2, :])
            nc.scalar.dma_start(xb_sb[:, KD // 2:KD, :], xb[:, KD // 2:KD, :])
            grep_sb = cpool.tile([P, C], F32)

            hb_sb = cpool.tile([P, KF, KB], BF16)
            hf_sb = cpool.tile([P, KF, KF8], FP8)

            # phase 1: h = gelu(x @ w1); bf16 tile then fp8 tile per
            # f-chunk; w1 streams in two-f-chunk transfers to halve the
            # completion-event count.
            for f in range(KF):
                if f % 2 == 0:
                    w1b_t = w1bpool.tile([P, 2, KD, P], BF16, tag="w1b")
                    nc.gpsimd.dma_start(w1b_t[:], w1b[f // 2])
                    if f == 0:
                        w1f_t = w1f_t0
                    else:
                        w1f_t = w1fpool.tile([P, 2, KD // 2, 2, P], FP8,
                                             tag="w1f")
                        nc.sync.dma_start(w1f_t[:], w1f[f // 2])
                if f == 6:
                    nc.scalar.dma_start(grep_sb[:], grep[:])
                fi = f % 2
                for t in range(NF):
                    ps = phf.tile([P, 512], F32)
                    for kk in range(KD // 2):
                        nc.tensor.matmul(
                            ps[:],
                            lhsT=w1f_t[:, fi, kk, :, :],
                            rhs=xf_sb[:, 2 * kk:2 * kk + 2, ts(t, 512)],
                            start=(kk == 0),
                            stop=(kk == KD // 2 - 1),
                            perf_mode=DR,
                        )
                    nc.scalar.activation(
                        hf_sb[:, f, ts(t, 512)], ps[:],
                        mybir.ActivationFunctionType.Gelu,
                        scale=1.0 / (SX * SW),
                    )
                for t in range(NB):
                    ps = phb.tile([P, 512], F32)
                    for k in range(KD):
                        nc.tensor.matmul(
                            ps[:],
                            lhsT=w1b_t[:, fi, k, :],
                            rhs=xb_sb[:, k, ts(t, 512)],
                            start=(k == 0),
                            stop=(k == KD - 1),
                        )
                    nc.scalar.activation(
                        hb_sb[:, f, ts(t, 512)], ps[:],
                        mybir.ActivationFunctionType.Gelu,
                    )

            # phase 2: yT[d, c] = gate[c] * sum_f w2[f, d] * h[f, c]
            for d in range(KD):
                w2b_t = w2bpool.tile([P, KF, P], BF16, tag="w2b")
                nc.gpsimd.dma_start(w2b_t[:], w2b[d])
                w2f_t = w2fpool.tile([P, KF // 2, 2, P], FP8, tag="w2f")
                nc.gpsimd.dma_start(w2f_t[:], w2f[d])
                for t in range(NB):
                    ps = pyb.tile([P, 512], F32)
                    for k in range(KF):
                        nc.tensor.matmul(
                            ps[:],
                            lhsT=w2b_t[:, k, :],
                            rhs=hb_sb[:, k, ts(t, 512)],
                            start=(k == 0),
                            stop=(k == KF - 1),
                        )
                    y_sb = ypool.tile([P, 512], F32, tag="yb")
                    nc.vector.tensor_mul(
                        out=y_sb[:], in0=ps[:], in1=grep_sb[:, ts(t, 512)]
                    )
                    nc.sync.dma_start(yT[ts(d, P), ts(t, 512)], y_sb[:])
                for t in range(NF):
                    ps = pyf.tile([P, 512], F32)
                    for kk in range(KF // 2):
                        nc.tensor.matmul(
                            ps[:],
                            lhsT=w2f_t[:, kk, :, :],
                            rhs=hf_sb[:, 2 * kk:2 * kk + 2, ts(t, 512)],
                            start=(kk == 0),
                            stop=(kk == KF // 2 - 1),
                            perf_mode=DR,
                        )
                    y_sb = ypool.tile([P, 512], F32, tag="yf")
                    nc.vector.tensor_mul(
                        out=y_sb[:], in0=ps[:],
                        in1=grep_sb[:, KB + t * 512:KB + (t + 1) * 512],
                    )
                    nc.sync.dma_start(
                        yT[ts(d, P), KB + t * 512:KB + (t + 1) * 512], y_sb[:]
                    )
    return _legalize_waits(nc)


_BUILT = {}


def _get_router():
    if "router" not in _BUILT:
        _BUILT["router"] = build_router()
    return _BUILT["router"]


def _get_mlp(KB, KF8):
    key = ("mlp", KB, KF8)
    if key not in _BUILT:
        _BUILT[key] = build_mlp_mixed(KB, KF8)
    return _BUILT[key]


def _run(name, nc, in_maps):
    kw = {}
    if PROFILE:
        kw["trace"] = True
    res = run_bass_kernel_spmd(nc, in_maps, core_ids=list(range(N_CORES)), **kw)
    if PROFILE:
        LAST_EXEC_NS[name] = res.exec_time_ns
        LAST_TRACE_DIRS[name] = getattr(res, "profile_json", None)
    return res.results


# ---------------------------------------------------------------------------
# host-side packing helpers
# ---------------------------------------------------------------------------
def _part3(a, np_dt):
    """[K*P, N] -> [P, K, N] with partition dim first (contiguous)."""
    kp, n = a.shape
    k = kp // P
    return np.ascontiguousarray(
        a.reshape(k, P, n).transpose(1, 0, 2).astype(np_dt, copy=False)
    )


def _xT_pack(xg, np_dt):
    """[C, D] tokens -> [P, KD, C] (d-major, partition-first)."""
    return _part3(np.ascontiguousarray(xg.T), np_dt)


def kernel(x, router_w, router_b, w1, w2):
    x = np.asarray(x, dtype=np.float32)
    router_w = np.asarray(router_w, dtype=np.float32)
    router_b = np.asarray(router_b, dtype=np.float32)
    w1 = np.asarray(w1, dtype=np.float32)
    w2 = np.asarray(w2, dtype=np.float32)

    xf_ = x.reshape(NTOK, D)

    # ---- launch 1: router -------------------------------------------------
    rw_f = _part3(router_w, np.float32)                    # [128, 8, 9]
    rw_hi = rw_f.astype(NP_BF16)
    rw_lo = (rw_f - rw_hi.astype(np.float32)).astype(NP_BF16)
    rw2_h = np.ascontiguousarray(
        np.stack([rw_hi, rw_lo], axis=2))                  # [128, 8, 2, 9]
    rbT_h = np.ascontiguousarray(router_b.reshape(E + 1, 1))
    id9_h = np.eye(E + 1, dtype=np.float32)
    in_maps = []
    for c in range(N_CORES):
        xs = xf_[c * TPC:(c + 1) * TPC]                    # [512, 1024]
        xT_h = np.ascontiguousarray(xs.T).reshape(KD, P, TPC)  # [8, 128, 512]
        xh_h = xT_h.astype(NP_BF16)
        xl_h = (xT_h - xh_h.astype(np.float32)).astype(NP_BF16)
        in_maps.append({"xh": xh_h, "xl": xl_h, "rw2": rw2_h,
                        "rbT": rbT_h, "id9": id9_h})
    results = _run("router", _get_router(), in_maps)
    gates = np.concatenate(
        [np.asarray(r["gates"], dtype=np.float32) for r in results], axis=0
    )                                                      # [4096, 9]

    # ---- host all-to-all dispatch + precision split ----------------------
    idx = [np.nonzero(gates[:, e] > 0)[0] for e in range(E)]
    KB = 512
    # per-expert: the KB highest-gate assignments run in bf16; the rest fp8.
    b_idx, f_idx = [], []
    for e in range(E):
        ide = idx[e]
        r = np.argsort(-gates[ide, e], kind="stable")
        b_idx.append(ide[r[:KB]])
        f_idx.append(ide[r[KB:]])
    max_f = max((len(i) for i in f_idx), default=0)
    KF8 = max(512, ((max_f + 511) // 512) * 512)

    nc_mlp = _get_mlp(KB, KF8)
    in_maps = []
    for e in range(E):
        w1_bl = np.stack(
            [
                w1[e][:, f * P:(f + 1) * P].reshape(KD, P, P).transpose(1, 0, 2)
                for f in range(KF)
            ]
        )                                                  # [32, 128, 8, 128] f32
        w2_bl = np.stack(
            [
                w2[e][:, d * P:(d + 1) * P].reshape(KF, P, P).transpose(1, 0, 2)
                for d in range(KD)
            ]
        )                                                  # [8, 128, 32, 128] f32
        w1b_h = w1_bl.astype(NP_BF16).reshape(KF // 2, 2, P, KD, P) \
            .transpose(0, 2, 1, 3, 4)
        w2b_h = w2_bl.astype(NP_BF16)
        w1f_h = (w1_bl * SW).astype(NP_FP8) \
            .reshape(KF // 2, 2, P, KD // 2, 2, P).transpose(0, 2, 1, 3, 4, 5)
        w2f_h = (w2_bl * SW).astype(NP_FP8).reshape(KD, P, KF // 2, 2, P)

        ib, if8 = b_idx[e], f_idx[e]
        nb, nf = len(ib), len(if8)
        xg_b = np.zeros((KB, D), dtype=np.float32)
        xg_b[:nb] = xf_[ib]
        xg_f = np.zeros((KF8, D), dtype=np.float32)
        xg_f[:nf] = xf_[if8]
        g = np.zeros((KB + KF8,), dtype=np.float32)
        g[:nb] = gates[ib, e]
        g[KB:KB + nf] = gates[if8, e] / (SW)

        in_maps.append({
            "w1b": np.ascontiguousarray(w1b_h),
            "w1f": np.ascontiguousarray(w1f_h),
            "w2b": np.ascontiguousarray(w2b_h),
            "w2f": np.ascontiguousarray(w2f_h),
            "xb": _xT_pack(xg_b, NP_BF16),
            "xf": _xT_pack(xg_f * SX, NP_FP8),
            "grep": np.ascontiguousarray(np.broadcast_to(g, (P, KB + KF8))),
        })

    # ---- launch 2: expert MLP --------------------------------------------
    results = _run("mlp", nc_mlp, in_maps)

    # ---- host combine -----------------------------------------------------
    out = gates[:, E:E + 1] * xf_                          # dummy identity expert
    for e in range(E):
        yT = np.asarray(results[e]["yT"], dtype=np.float32)    # [1024, KB+KF8]
        nb, nf = len(b_idx[e]), len(f_idx[e])
        if nb:
            out[b_idx[e]] += yT[:, :nb].T
        if nf:
            out[f_idx[e]] += yT[:, KB:KB + nf].T
    return out.reshape(B, T, D).astype(np.float32)


# revision 23
# speedup vs baseline: 1.1809x; 1.0024x over previous
"""MoE (top-2 of 8 experts + dummy identity expert) on 8 NeuronCores.

Strategy (expert parallelism, per the sharding hint):
  Launch 1 (router, token-parallel): each core computes logits -> softmax
    -> top-2 gates for its 512-token shard, fully on device (fp32 so the
    top-2 selection matches the fp32 reference bit-for-bit).
  Host all-to-all "dispatch": compact token indices per expert from the
    device-computed gates, gather+transpose token activations.
  Launch 2 (expert MLP, expert-parallel, MIXED PRECISION): core e holds
    expert e's weights. Tokens are split by routing risk s=||gates||_2:
    the top KB tokens per expert run in bf16; the low-gate remainder runs
    in fp8(e4m3) with DoubleRow perf mode (2x PE throughput). Quantization
    error scales with the gate, so low-gate tokens tolerate fp8.
  Host "combine": scatter-add per-expert outputs + dummy-expert term.
"""

import math
import os
import sys

for _p in ("/opt/trn_rl_repo",):
    if _p not in sys.path:
        sys.path.insert(0, _p)

import numpy as np
import ml_dtypes

import concourse.bass as bass
import concourse.mybir as mybir
import concourse.tile as tile
from concourse.bass import ts
from concourse.bass_utils import run_bass_kernel_spmd

# ---------------------------------------------------------------------------
# This container's walrus accepts at most ONE sync-wait command per
# instruction. Tile can attach several (body instructions and the
# kernel-tail drain). Hoist excess waits onto same-engine NoOps inserted
# immediately before the offending instruction — semantically identical
# (waits are AND conditions evaluated in stream order).
# ---------------------------------------------------------------------------
_WAITS_PER_INST = 1
_legalize_counter = [0]


def _legalize_waits(nc):
    for f in nc.m.functions:
        for bb in f.blocks:
            insts = list(bb.instructions)
            out = []
            changed = False
            for inst in insts:
                si = inst.sync_info
                waits = list(si.on_wait) if si is not None and si.on_wait else []
                if len(waits) > _WAITS_PER_INST:
                    changed = True
                    for w in waits[:-_WAITS_PER_INST]:
                        _legalize_counter[0] += 1
                        out.append(
                            mybir.InstNoOp(
                                name=f"legwait-{_legalize_counter[0]}",
                                ins=[],
                                outs=[],
                                engine=inst.engine,
                                sync_info=mybir.SyncInfo(
                                    on_wait=[w], on_update=[]
                                ),
                            )
                        )
                    si.on_wait = waits[-_WAITS_PER_INST:]
                out.append(inst)
            if changed:
                bb.instructions = out
    return nc


# ---------------------------------------------------------------------------
# Problem constants (hardcoded per contract; inputs are fixed-shape).
# ---------------------------------------------------------------------------
N_CORES = 8
B, T, D, F, E = 2, 2048, 1024, 4096, 8
NTOK = B * T            # 4096 tokens
TPC = NTOK // N_CORES   # 512 tokens/core in the router launch
P = 128
KD = D // P             # 8 contraction chunks over D
KF = F // P             # 32 contraction chunks over F

F32 = mybir.dt.float32
BF16 = mybir.dt.bfloat16
FP8 = mybir.dt.float8e4
NP_BF16 = ml_dtypes.bfloat16
NP_FP8 = ml_dtypes.float8_e4m3

# fp8 scaling: pre-psum = (SX*x) @ (SW*w1) = SX*SW*pre; gelu scale undoes it.
SX = 8.0
SW = 64.0

PROFILE = False          # set True (from test.py) to collect NTFF exec times
LAST_EXEC_NS = {}        # launch name -> exec_time_ns (filled when PROFILE)
LAST_TRACE_DIRS = {}


# ---------------------------------------------------------------------------
# Launch 1: router. Per core: 512 tokens -> gates [512, 9].
# ---------------------------------------------------------------------------
def build_router():
    NE = E + 1
    TH = TPC // 2   # 256-token half: two independent pipelines so the
    TTH = TH // P   # first half's softmax chain overlaps the second
    nc = bass.Bass()
    # x and router weights arrive as bf16 hi/lo pairs; computing
    #   x_hi@rw_hi + x_lo@rw_hi + x_hi@rw_lo
    # in fp32 PSUM reproduces fp32 logits to ~1e-5 (verified: every token's
    # error is <= 10% of its top-2 decision gap, so selection is exact)
    # while running the PE at 1 cycle/row instead of fp32's 4. Each token's
    # accumulation order is unchanged by the half-split, so gates are
    # bit-identical to the unsplit version.
    xh = nc.declare_dram_parameter("xh", [KD, P, TPC], BF16, isOutput=False)
    xl = nc.declare_dram_parameter("xl", [KD, P, TPC], BF16, isOutput=False)
    rw2 = nc.declare_dram_parameter("rw2", [P, KD, 2, NE], BF16, isOutput=False)
    rbT = nc.declare_dram_parameter("rbT", [NE, 1], F32, isOutput=False)
    id9 = nc.declare_dram_parameter("id9", [NE, NE], F32, isOutput=False)
    gates = nc.declare_dram_parameter("gates", [2, TH, NE], F32, isOutput=True)

    with tile.TileContext(nc) as tc:
        with (
            tc.tile_pool(name="const", bufs=1) as cpool,
            tc.tile_pool(name="xp", bufs=8) as xpool,
            tc.tile_pool(name="work", bufs=2) as pool,
            tc.tile_pool(name="psum", bufs=1, space="PSUM") as pp,
            tc.tile_pool(name="psum2", bufs=1, space="PSUM") as pp2,
        ):
            rw_sb = cpool.tile([P, KD, 2, NE], BF16)
            nc.sync.dma_start(rw_sb[:], rw2[:])
            rbT_sb = cpool.tile([NE, 1], F32)
            id9_sb = cpool.tile([NE, NE], F32)

            dma_engines = [nc.sync, nc.gpsimd, nc.scalar]
            qi = [0]

            def nxt():
                e = dma_engines[qi[0] % 3]
                qi[0] += 1
                return e

            for half in range(2):
                tok = ts(half, TH)
                ps_lgT = pp2.tile([NE, TH], F32, name=f"pslg{half}")
                for k in range(KD):
                    xh_sb = xpool.tile([P, TH], BF16, tag=f"xh{half}")
                    nxt().dma_start(xh_sb[:], xh[k][:, tok])
                    xl_sb = xpool.tile([P, TH], BF16, tag=f"xl{half}")
                    nxt().dma_start(xl_sb[:], xl[k][:, tok])
                    if half == 0 and k == 1:
                        nc.scalar.dma_start(rbT_sb[:], rbT[:])
                        nc.scalar.dma_start(id9_sb[:], id9[:])
                    nc.tensor.matmul(
                        ps_lgT[:], lhsT=rw_sb[:, k, 0, :], rhs=xh_sb[:],
                        start=(k == 0), stop=False,
                    )
                    nc.tensor.matmul(
                        ps_lgT[:], lhsT=rw_sb[:, k, 0, :], rhs=xl_sb[:],
                        start=False, stop=False,
                    )
                    nc.tensor.matmul(
                        ps_lgT[:], lhsT=rw_sb[:, k, 1, :], rhs=xh_sb[:],
                        start=False, stop=(k == KD - 1),
                    )
                lgT_sb = pool.tile([NE, TH], F32, tag=f"lgT{half}")
                nc.vector.tensor_tensor(
                    lgT_sb[:], ps_lgT[:],
                    rbT_sb[:].to_broadcast([NE, TH]),
                    mybir.AluOpType.add,
                )
                # transpose back to token-major [128, TTH, 9] via PE
                ps = pp.tile([P, TTH, NE], F32, name=f"pst{half}")
                for tt in range(TTH):
                    nc.tensor.transpose(
                        ps[:, tt, :], lgT_sb[:, ts(tt, P)], id9_sb[:]
                    )

                # logits are O(+-5): exp() cannot overflow fp32, and the
                # softmax normalization cancels any shift, so skip the max-
                # subtraction entirely (selection is order-preserving).
                sh3 = [P, TTH, NE]
                e_sb = pool.tile(sh3, F32, tag=f"e{half}")
                nc.scalar.activation(
                    e_sb[:], ps[:], mybir.ActivationFunctionType.Exp,
                )
                s = pool.tile([P, TTH], F32, tag=f"s{half}")
                nc.vector.tensor_reduce(
                    s[:], e_sb[:], axis=mybir.AxisListType.X,
                    op=mybir.AluOpType.add,
                )
                mx = pool.tile([P, TTH], F32, tag=f"mx{half}")
                nc.vector.tensor_reduce(
                    mx[:], e_sb[:], axis=mybir.AxisListType.X,
                    op=mybir.AluOpType.max,
                )
                # knock out the top-1, take max again -> second-largest
                mlt = pool.tile(sh3, F32, tag=f"mlt{half}")
                nc.vector.tensor_tensor(
                    mlt[:], e_sb[:], mx[:, :, None].to_broadcast(sh3),
                    mybir.AluOpType.is_lt,
                )
                emask = pool.tile(sh3, F32, tag=f"emask{half}")
                nc.vector.tensor_mul(out=emask[:], in0=mlt[:], in1=e_sb[:])
                m2 = pool.tile([P, TTH], F32, tag=f"m2{half}")
                nc.vector.tensor_reduce(
                    m2[:], emask[:], axis=mybir.AxisListType.X,
                    op=mybir.AluOpType.max,
                )
                gmask = pool.tile(sh3, F32, tag=f"gmask{half}")
                nc.vector.tensor_tensor(
                    gmask[:], e_sb[:], m2[:, :, None].to_broadcast(sh3),
                    mybir.AluOpType.is_ge,
                )
                gsel = pool.tile(sh3, F32, tag=f"gsel{half}")
                nc.vector.tensor_mul(out=gsel[:], in0=gmask[:], in1=e_sb[:])
                rs = pool.tile([P, TTH], F32, tag=f"rs{half}")
                nc.vector.reciprocal(rs[:], s[:])
                gfin = pool.tile(sh3, F32, tag=f"gfin{half}")
                nc.vector.tensor_tensor(
                    gfin[:], gsel[:], rs[:, :, None].to_broadcast(sh3),
                    mybir.AluOpType.mult,
                )
                nc.sync.dma_start(
                    gates[half].rearrange("(tt p) e -> p tt e", p=P), gfin[:]
                )
    return _legalize_waits(nc)


# ---------------------------------------------------------------------------
# Launch 2: mixed-precision expert MLP. Per core: KB bf16 tokens + KF8 fp8
# tokens through one expert.
#   yT[d, c] = gate[c] * (gelu(x @ w1) @ w2)[c, d]   (transposed output)
# fp8 tiles use DoubleRow perf mode: lhsT [128,2,128] x rhs [128,2,512]
# contracts 256 elements per instruction at 2x bf16 throughput.
# ---------------------------------------------------------------------------
def build_mlp_mixed(KB, KF8):
    assert KB % 512 == 0 and KF8 % 512 == 0
    NB = KB // 512
    NF = KF8 // 512
    C = KB + KF8
    DR = mybir.MatmulPerfMode.DoubleRow
    nc = bass.Bass()
    w1b = nc.declare_dram_parameter("w1b", [KF // 2, P, 2, KD, P], BF16,
                                    isOutput=False)
    w1f = nc.declare_dram_parameter("w1f", [KF // 2, P, 2, KD // 2, 2, P], FP8,
                                    isOutput=False)
    w2b = nc.declare_dram_parameter("w2b", [KD, P, KF, P], BF16, isOutput=False)
    w2f = nc.declare_dram_parameter("w2f", [KD, P, KF // 2, 2, P], FP8,
                                    isOutput=False)
    xb = nc.declare_dram_parameter("xb", [P, KD, KB], BF16, isOutput=False)
    xf = nc.declare_dram_parameter("xf", [P, KD, KF8], FP8, isOutput=False)
    grep = nc.declare_dram_parameter("grep", [P, C], F32, isOutput=False)
    yT = nc.declare_dram_parameter("yT", [D, C], F32, isOutput=True)

    with tile.TileContext(nc) as tc:
        with (
            tc.tile_pool(name="const", bufs=1) as cpool,
            tc.tile_pool(name="w1bp", bufs=3) as w1bpool,
            tc.tile_pool(name="w1fp", bufs=3) as w1fpool,
            tc.tile_pool(name="w2bp", bufs=2) as w2bpool,
            tc.tile_pool(name="w2fp", bufs=2) as w2fpool,
            tc.tile_pool(name="yp", bufs=4) as ypool,
            tc.tile_pool(name="ph_b", bufs=2, space="PSUM") as phb,
            tc.tile_pool(name="ph_f", bufs=2, space="PSUM") as phf,
            tc.tile_pool(name="py_b", bufs=2, space="PSUM") as pyb,
            tc.tile_pool(name="py_f", bufs=2, space="PSUM") as pyf,
        ):
            # token activations: split the bf16 x DMA over two queues so the
            # first phase-1 group can start ASAP.
            xf_sb = cpool.tile([P, KD, KF8], FP8)
            nc.scalar.dma_start(xf_sb[:, 0:KD // 2, :], xf[:, 0:KD // 2, :])
            nc.gpsimd.dma_start(xf_sb[:, KD // 2:KD, :], xf[:, KD // 2:KD, :])
            w1f_t0 = w1fpool.tile([P, 2, KD // 2, 2, P], FP8, tag="w1f")
            nc.sync.dma_start(w1f_t0[:], w1f[0])
            xb_sb = cpool.tile([P, KD, KB], BF16)
            nc.sync.dma_start(xb_sb[:, 0:KD // 2, :], xb[:, 0:KD // 2, :])
            nc.scalar.dma_start(xb_sb[:, KD // 2:KD, :], xb[:, KD // 2:KD, :])
            grep_sb = cpool.tile([P, C], F32)

            hb_sb = cpool.tile([P, KF, KB], BF16)
            hf_sb = cpool.tile([P, KF, KF8], FP8)

            # phase 1: h = gelu(x @ w1); bf16 tile then fp8 tile per
            # f-chunk; w1 streams in two-f-chunk transfers to halve the
            # completion-event count.
            for f in range(KF):
                if f % 2 == 0:
                    w1b_t = w1bpool.tile([P, 2, KD, P], BF16, tag="w1b")
                    nc.gpsimd.dma_start(w1b_t[:], w1b[f // 2])
                    if f == 0:
                        w1f_t = w1f_t0
                    else:
                        w1f_t = w1fpool.tile([P, 2, KD // 2, 2, P], FP8,
                                             tag="w1f")
                        nc.sync.dma_start(w1f_t[:], w1f[f // 2])
                if f == 6:
                    nc.scalar.dma_start(grep_sb[:], grep[:])
                fi = f % 2
                for t in range(NF):
                    ps = phf.tile([P, 512], F32)
                    for kk in range(KD // 2):
                        nc.tensor.matmul(
                            ps[:],
                            lhsT=w1f_t[:, fi, kk, :, :],
                            rhs=xf_sb[:, 2 * kk:2 * kk + 2, ts(t, 512)],
                            start=(kk == 0),
                            stop=(kk == KD // 2 - 1),
                            perf_mode=DR,
                        )
                    nc.scalar.activation(
                        hf_sb[:, f, ts(t, 512)], ps[:],
                        mybir.ActivationFunctionType.Gelu,
                        scale=1.0 / (SX * SW),
                    )
                for t in range(NB):
                    ps = phb.tile([P, 512], F32)
                    for k in range(KD):
                        nc.tensor.matmul(
                            ps[:],
                            lhsT=w1b_t[:, fi, k, :],
                            rhs=xb_sb[:, k, ts(t, 512)],
                            start=(k == 0),
                            stop=(k == KD - 1),
                        )
                    nc.scalar.activation(
                        hb_sb[:, f, ts(t, 512)], ps[:],
                        mybir.ActivationFunctionType.Gelu,
                    )

            # phase 2: yT[d, c] = gate[c] * sum_f w2[f, d] * h[f, c]
            for d in range(KD):
                w2b_t = w2bpool.tile([P, KF, P], BF16, tag="w2b")
                nc.gpsimd.dma_start(w2b_t[:], w2b[d])
                w2f_t = w2fpool.tile([P, KF // 2, 2, P], FP8, tag="w2f")
                nc.gpsimd.dma_start(w2f_t[:], w2f[d])
                for t in range(NB):
                    ps = pyb.tile([P, 512], F32)
                    for k in range(KF):
                        nc.tensor.matmul(
                            ps[:],
                            lhsT=w2b_t[:, k, :],
                            rhs=hb_sb[:, k, ts(t, 512)],
                            start=(k == 0),
                            stop=(k == KF - 1),
                        )
                    y_sb = ypool.tile([P, 512], F32, tag="yb")
                    nc.vector.tensor_mul(
                        out=y_sb[:], in0=ps[:], in1=grep_sb[:, ts(t, 512)]
                    )
                    nc.sync.dma_start(yT[ts(d, P), ts(t, 512)], y_sb[:])
                for t in range(NF):
                    ps = pyf.tile([P, 512], F32)
                    for kk in range(KF // 2):
                        nc.tensor.matmul(
                            ps[:],
                            lhsT=w2f_t[:, kk, :, :],
                            rhs=hf_sb[:, 2 * kk:2 * kk + 2, ts(t, 512)],
                            start=(kk == 0),
                            stop=(kk == KF // 2 - 1),
                            perf_mode=DR,
                        )
                    y_sb = ypool.tile([P, 512], F32, tag="yf")
                    nc.vector.tensor_mul(
                        out=y_sb[:], in0=ps[:],
                        in1=grep_sb[:, KB + t * 512:KB + (t + 1) * 512],
                    )
                    nc.sync.dma_start(
                        yT[ts(d, P), KB + t * 512:KB + (t + 1) * 512], y_sb[:]
                    )
    return _legalize_waits(nc)


_BUILT = {}


def _get_router():
    if "router" not in _BUILT:
        _BUILT["router"] = build_router()
    return _BUILT["router"]


def _get_mlp(KB, KF8):
    key = ("mlp", KB, KF8)
    if key not in _BUILT:
        _BUILT[key] = build_mlp_mixed(KB, KF8)
    return _BUILT[key]


def _run(name, nc, in_maps):
    kw = {}
    if PROFILE:
        kw["trace"] = True
    res = run_bass_kernel_spmd(nc, in_maps, core_ids=list(range(N_CORES)), **kw)
    if PROFILE:
        LAST_EXEC_NS[name] = res.exec_time_ns
        LAST_TRACE_DIRS[name] = getattr(res, "profile_json", None)
    return res.results


# ---------------------------------------------------------------------------
# host-side packing helpers
# ---------------------------------------------------------------------------
def _part3(a, np_dt):
    """[K*P, N] -> [P, K, N] with partition dim first (contiguous)."""
    kp, n = a.shape
    k = kp // P
    return np.ascontiguousarray(
        a.reshape(k, P, n).transpose(1, 0, 2).astype(np_dt, copy=False)
    )


def _xT_pack(xg, np_dt):
    """[C, D] tokens -> [P, KD, C] (d-major, partition-first)."""
    return _part3(np.ascontiguousarray(xg.T), np_dt)


def kernel(x, router_w, router_b, w1, w2):
    x = np.asarray(x, dtype=np.float32)
    router_w = np.asarray(router_w, dtype=np.float32)
    router_b = np.asarray(router_b, dtype=np.float32)
    w1 = np.asarray(w1, dtype=np.float32)
    w2 = np.asarray(w2, dtype=np.float32)

    xf_ = x.reshape(NTOK, D)

    # ---- launch 1: router -------------------------------------------------
    rw_f = _part3(router_w, np.float32)                    # [128, 8, 9]
    rw_hi = rw_f.astype(NP_BF16)
    rw_lo = (rw_f - rw_hi.astype(np.float32)).astype(NP_BF16)
    rw2_h = np.ascontiguousarray(
        np.stack([rw_hi, rw_lo], axis=2))                  # [128, 8, 2, 9]
    rbT_h = np.ascontiguousarray(router_b.reshape(E + 1, 1))
    id9_h = np.eye(E + 1, dtype=np.float32)
    in_maps = []
    for c in range(N_CORES):
        xs = xf_[c * TPC:(c + 1) * TPC]                    # [512, 1024]
        xT_h = np.ascontiguousarray(xs.T).reshape(KD, P, TPC)  # [8, 128, 512]
        xh_h = xT_h.astype(NP_BF16)
        xl_h = (xT_h - xh_h.astype(np.float32)).astype(NP_BF16)
        in_maps.append({"xh": xh_h, "xl": xl_h, "rw2": rw2_h,
                        "rbT": rbT_h, "id9": id9_h})
    results = _run("router", _get_router(), in_maps)
    gates = np.concatenate(
        [np.asarray(r["gates"], dtype=np.float32).reshape(TPC, E + 1)
         for r in results], axis=0
    )                                                      # [4096, 9]

    # ---- host all-to-all dispatch + precision split ----------------------
    idx = [np.nonzero(gates[:, e] > 0)[0] for e in range(E)]
    KB = 512
    # per-expert: the KB highest-gate assignments run in bf16; the rest fp8.
    b_idx, f_idx = [], []
    for e in range(E):
        ide = idx[e]
        r = np.argsort(-gates[ide, e], kind="stable")
        b_idx.append(ide[r[:KB]])
        f_idx.append(ide[r[KB:]])
    max_f = max((len(i) for i in f_idx), default=0)
    KF8 = max(512, ((max_f + 511) // 512) * 512)

    nc_mlp = _get_mlp(KB, KF8)
    in_maps = []
    for e in range(E):
        w1_bl = np.stack(
            [
                w1[e][:, f * P:(f + 1) * P].reshape(KD, P, P).transpose(1, 0, 2)
                for f in range(KF)
            ]
        )                                                  # [32, 128, 8, 128] f32
        w2_bl = np.stack(
            [
                w2[e][:, d * P:(d + 1) * P].reshape(KF, P, P).transpose(1, 0, 2)
                for d in range(KD)
            ]
        )                                                  # [8, 128, 32, 128] f32
        w1b_h = w1_bl.astype(NP_BF16).reshape(KF // 2, 2, P, KD, P) \
            .transpose(0, 2, 1, 3, 4)
        w2b_h = w2_bl.astype(NP_BF16)
        w1f_h = (w1_bl * SW).astype(NP_FP8) \
            .reshape(KF // 2, 2, P, KD // 2, 2, P).transpose(0, 2, 1, 3, 4, 5)
        w2f_h = (w2_bl * SW).astype(NP_FP8).reshape(KD, P, KF // 2, 2, P)

        ib, if8 = b_idx[e], f_idx[e]
        nb, nf = len(ib), len(if8)
        xg_b = np.zeros((KB, D), dtype=np.float32)
        xg_b[:nb] = xf_[ib]
        xg_f = np.zeros((KF8, D), dtype=np.float32)
        xg_f[:nf] = xf_[if8]
        g = np.zeros((KB + KF8,), dtype=np.float32)
        g[:nb] = gates[ib, e]
        g[KB:KB + nf] = gates[if8, e] / (SW)

        in_maps.append({
            "w1b": np.ascontiguousarray(w1b_h),
            "w1f": np.ascontiguousarray(w1f_h),
            "w2b": np.ascontiguousarray(w2b_h),
            "w2f": np.ascontiguousarray(w2f_h),
            "xb": _xT_pack(xg_b, NP_BF16),
            "xf": _xT_pack(xg_f * SX, NP_FP8),
            "grep": np.ascontiguousarray(np.broadcast_to(g, (P, KB + KF8))),
        })

    # ---- launch 2: expert MLP --------------------------------------------
    results = _run("mlp", nc_mlp, in_maps)

    # ---- host combine -----------------------------------------------------
    out = gates[:, E:E + 1] * xf_                          # dummy identity expert
    for e in range(E):
        yT = np.asarray(results[e]["yT"], dtype=np.float32)    # [1024, KB+KF8]
        nb, nf = len(b_idx[e]), len(f_idx[e])
        if nb:
            out[b_idx[e]] += yT[:, :nb].T
        if nf:
            out[f_idx[e]] += yT[:, KB:KB + nf].T
    return out.reshape(B, T, D).astype(np.float32)


# revision 24
# speedup vs baseline: 1.1837x; 1.0024x over previous
"""MoE (top-2 of 8 experts + dummy identity expert) on 8 NeuronCores.

Strategy (expert parallelism, per the sharding hint):
  Launch 1 (router, token-parallel): each core computes logits -> softmax
    -> top-2 gates for its 512-token shard, fully on device (fp32 so the
    top-2 selection matches the fp32 reference bit-for-bit).
  Host all-to-all "dispatch": compact token indices per expert from the
    device-computed gates, gather+transpose token activations.
  Launch 2 (expert MLP, expert-parallel, MIXED PRECISION): core e holds
    expert e's weights. Tokens are split by routing risk s=||gates||_2:
    the top KB tokens per expert run in bf16; the low-gate remainder runs
    in fp8(e4m3) with DoubleRow perf mode (2x PE throughput). Quantization
    error scales with the gate, so low-gate tokens tolerate fp8.
  Host "combine": scatter-add per-expert outputs + dummy-expert term.
"""

import math
import os
import sys

for _p in ("/opt/trn_rl_repo",):
    if _p not in sys.path:
        sys.path.insert(0, _p)

import numpy as np
import ml_dtypes

import concourse.bass as bass
import concourse.mybir as mybir
import concourse.tile as tile
from concourse.bass import ts
from concourse.bass_utils import run_bass_kernel_spmd

# ---------------------------------------------------------------------------
# This container's walrus accepts at most ONE sync-wait command per
# instruction. Tile can attach several (body instructions and the
# kernel-tail drain). Hoist excess waits onto same-engine NoOps inserted
# immediately before the offending instruction — semantically identical
# (waits are AND conditions evaluated in stream order).
# ---------------------------------------------------------------------------
_WAITS_PER_INST = 1
_legalize_counter = [0]


def _legalize_waits(nc):
    for f in nc.m.functions:
        for bb in f.blocks:
            insts = list(bb.instructions)
            out = []
            changed = False
            for inst in insts:
                si = inst.sync_info
                waits = list(si.on_wait) if si is not None and si.on_wait else []
                if len(waits) > _WAITS_PER_INST:
                    changed = True
                    for w in waits[:-_WAITS_PER_INST]:
                        _legalize_counter[0] += 1
                        out.append(
                            mybir.InstNoOp(
                                name=f"legwait-{_legalize_counter[0]}",
                                ins=[],
                                outs=[],
                                engine=inst.engine,
                                sync_info=mybir.SyncInfo(
                                    on_wait=[w], on_update=[]
                                ),
                            )
                        )
                    si.on_wait = waits[-_WAITS_PER_INST:]
                out.append(inst)
            if changed:
                bb.instructions = out
    return nc


# ---------------------------------------------------------------------------
# Problem constants (hardcoded per contract; inputs are fixed-shape).
# ---------------------------------------------------------------------------
N_CORES = 8
B, T, D, F, E = 2, 2048, 1024, 4096, 8
NTOK = B * T            # 4096 tokens
TPC = NTOK // N_CORES   # 512 tokens/core in the router launch
P = 128
KD = D // P             # 8 contraction chunks over D
KF = F // P             # 32 contraction chunks over F

F32 = mybir.dt.float32
BF16 = mybir.dt.bfloat16
FP8 = mybir.dt.float8e4
NP_BF16 = ml_dtypes.bfloat16
NP_FP8 = ml_dtypes.float8_e4m3

# fp8 scaling: pre-psum = (SX*x) @ (SW*w1) = SX*SW*pre; gelu scale undoes it.
SX = 8.0
SW = 64.0

PROFILE = False          # set True (from test.py) to collect NTFF exec times
LAST_EXEC_NS = {}        # launch name -> exec_time_ns (filled when PROFILE)
LAST_TRACE_DIRS = {}


# ---------------------------------------------------------------------------
# Launch 1: router. Per core: 512 tokens -> gates [512, 9].
# ---------------------------------------------------------------------------
def build_router():
    NE = E + 1
    TH = TPC // 2   # 256-token half: two independent pipelines so the
    TTH = TH // P   # first half's softmax chain overlaps the second
    nc = bass.Bass()
    # x and router weights arrive as bf16 hi/lo pairs; computing
    #   x_hi@rw_hi + x_lo@rw_hi + x_hi@rw_lo
    # in fp32 PSUM reproduces fp32 logits to ~1e-5 (verified: every token's
    # error is <= 10% of its top-2 decision gap, so selection is exact)
    # while running the PE at 1 cycle/row instead of fp32's 4. Each token's
    # accumulation order is unchanged by the half-split, so gates are
    # bit-identical to the unsplit version.
    xh = nc.declare_dram_parameter("xh", [KD, P, TPC], BF16, isOutput=False)
    xl = nc.declare_dram_parameter("xl", [KD, P, TPC], BF16, isOutput=False)
    rw2 = nc.declare_dram_parameter("rw2", [P, KD, 2, NE], BF16, isOutput=False)
    rbT = nc.declare_dram_parameter("rbT", [NE, 1], F32, isOutput=False)
    id9 = nc.declare_dram_parameter("id9", [NE, NE], F32, isOutput=False)
    gates = nc.declare_dram_parameter("gates", [2, TH, NE], F32, isOutput=True)

    with tile.TileContext(nc) as tc:
        with (
            tc.tile_pool(name="const", bufs=1) as cpool,
            tc.tile_pool(name="xp", bufs=8) as xpool,
            tc.tile_pool(name="work", bufs=2) as pool,
            tc.tile_pool(name="psum", bufs=1, space="PSUM") as pp,
            tc.tile_pool(name="psum2", bufs=1, space="PSUM") as pp2,
        ):
            rw_sb = cpool.tile([P, KD, 2, NE], BF16)
            nc.sync.dma_start(rw_sb[:], rw2[:])
            rbT_sb = cpool.tile([NE, 1], F32)
            id9_sb = cpool.tile([NE, NE], F32)

            dma_engines = [nc.sync, nc.gpsimd, nc.scalar]
            qi = [0]

            def nxt():
                e = dma_engines[qi[0] % 3]
                qi[0] += 1
                return e

            for half in range(2):
                tok = ts(half, TH)
                ps_lgT = pp2.tile([NE, TH], F32, name=f"pslg{half}")
                for k in range(KD):
                    xh_sb = xpool.tile([P, TH], BF16, tag=f"xh{half}")
                    nxt().dma_start(xh_sb[:], xh[k][:, tok])
                    xl_sb = xpool.tile([P, TH], BF16, tag=f"xl{half}")
                    nxt().dma_start(xl_sb[:], xl[k][:, tok])
                    if half == 0 and k == 1:
                        nc.scalar.dma_start(rbT_sb[:], rbT[:])
                        nc.scalar.dma_start(id9_sb[:], id9[:])
                    nc.tensor.matmul(
                        ps_lgT[:], lhsT=rw_sb[:, k, 0, :], rhs=xh_sb[:],
                        start=(k == 0), stop=False,
                    )
                    nc.tensor.matmul(
                        ps_lgT[:], lhsT=rw_sb[:, k, 0, :], rhs=xl_sb[:],
                        start=False, stop=False,
                    )
                    nc.tensor.matmul(
                        ps_lgT[:], lhsT=rw_sb[:, k, 1, :], rhs=xh_sb[:],
                        start=False, stop=(k == KD - 1),
                    )
                lgT_sb = pool.tile([NE, TH], F32, tag=f"lgT{half}")
                nc.vector.tensor_tensor(
                    lgT_sb[:], ps_lgT[:],
                    rbT_sb[:].to_broadcast([NE, TH]),
                    mybir.AluOpType.add,
                )
                # transpose back to token-major [128, TTH, 9] via PE
                ps = pp.tile([P, TTH, NE], F32, name=f"pst{half}")
                for tt in range(TTH):
                    nc.tensor.transpose(
                        ps[:, tt, :], lgT_sb[:, ts(tt, P)], id9_sb[:]
                    )

                # logits are O(+-5): exp() cannot overflow fp32, and the
                # softmax normalization cancels any shift, so skip the max-
                # subtraction entirely (selection is order-preserving).
                sh3 = [P, TTH, NE]
                e_sb = pool.tile(sh3, F32, tag=f"e{half}")
                nc.scalar.activation(
                    e_sb[:], ps[:], mybir.ActivationFunctionType.Exp,
                )
                s = pool.tile([P, TTH], F32, tag=f"s{half}")
                nc.vector.tensor_reduce(
                    s[:], e_sb[:], axis=mybir.AxisListType.X,
                    op=mybir.AluOpType.add,
                )
                mx = pool.tile([P, TTH], F32, tag=f"mx{half}")
                nc.vector.tensor_reduce(
                    mx[:], e_sb[:], axis=mybir.AxisListType.X,
                    op=mybir.AluOpType.max,
                )
                # knock out the top-1, take max again -> second-largest
                mlt = pool.tile(sh3, F32, tag=f"mlt{half}")
                nc.vector.tensor_tensor(
                    mlt[:], e_sb[:], mx[:, :, None].to_broadcast(sh3),
                    mybir.AluOpType.is_lt,
                )
                emask = pool.tile(sh3, F32, tag=f"emask{half}")
                nc.vector.tensor_mul(out=emask[:], in0=mlt[:], in1=e_sb[:])
                m2 = pool.tile([P, TTH], F32, tag=f"m2{half}")
                nc.vector.tensor_reduce(
                    m2[:], emask[:], axis=mybir.AxisListType.X,
                    op=mybir.AluOpType.max,
                )
                gmask = pool.tile(sh3, F32, tag=f"gmask{half}")
                nc.vector.tensor_tensor(
                    gmask[:], e_sb[:], m2[:, :, None].to_broadcast(sh3),
                    mybir.AluOpType.is_ge,
                )
                gsel = pool.tile(sh3, F32, tag=f"gsel{half}")
                nc.vector.tensor_mul(out=gsel[:], in0=gmask[:], in1=e_sb[:])
                rs = pool.tile([P, TTH], F32, tag=f"rs{half}")
                nc.vector.reciprocal(rs[:], s[:])
                gfin = pool.tile(sh3, F32, tag=f"gfin{half}")
                nc.vector.tensor_tensor(
                    gfin[:], gsel[:], rs[:, :, None].to_broadcast(sh3),
                    mybir.AluOpType.mult,
                )
                nc.sync.dma_start(
                    gates[half].rearrange("(tt p) e -> p tt e", p=P), gfin[:]
                )
    return _legalize_waits(nc)


# ---------------------------------------------------------------------------
# Launch 2: mixed-precision expert MLP. Per core: KB bf16 tokens + KF8 fp8
# tokens through one expert.
#   yT[d, c] = gate[c] * (gelu(x @ w1) @ w2)[c, d]   (transposed output)
# fp8 tiles use DoubleRow perf mode: lhsT [128,2,128] x rhs [128,2,512]
# contracts 256 elements per instruction at 2x bf16 throughput.
# ---------------------------------------------------------------------------
def build_mlp_mixed(KB, KF8):
    assert KB % 512 == 0 and KF8 % 512 == 0
    NB = KB // 512
    NF = KF8 // 512
    C = KB + KF8
    DR = mybir.MatmulPerfMode.DoubleRow
    nc = bass.Bass()
    w1b = nc.declare_dram_parameter("w1b", [KF // 2, P, 2, KD, P], BF16,
                                    isOutput=False)
    w1f = nc.declare_dram_parameter("w1f", [KF // 2, P, 2, KD // 2, 2, P], FP8,
                                    isOutput=False)
    w2b = nc.declare_dram_parameter("w2b", [KD, P, KF, P], BF16, isOutput=False)
    w2f = nc.declare_dram_parameter("w2f", [KD, P, KF // 2, 2, P], FP8,
                                    isOutput=False)
    xb = nc.declare_dram_parameter("xb", [P, KD, KB], BF16, isOutput=False)
    xf = nc.declare_dram_parameter("xf", [P, KD, KF8], FP8, isOutput=False)
    grep = nc.declare_dram_parameter("grep", [P, C], F32, isOutput=False)
    yT = nc.declare_dram_parameter("yT", [D, C], F32, isOutput=True)

    with tile.TileContext(nc) as tc:
        with (
            tc.tile_pool(name="const", bufs=1) as cpool,
            tc.tile_pool(name="w1bp", bufs=3) as w1bpool,
            tc.tile_pool(name="w1fp", bufs=3) as w1fpool,
            tc.tile_pool(name="w2bp", bufs=2) as w2bpool,
            tc.tile_pool(name="w2fp", bufs=2) as w2fpool,
            tc.tile_pool(name="yp", bufs=4) as ypool,
            tc.tile_pool(name="ph_b", bufs=2, space="PSUM") as phb,
            tc.tile_pool(name="ph_f", bufs=2, space="PSUM") as phf,
            tc.tile_pool(name="py_b", bufs=2, space="PSUM") as pyb,
            tc.tile_pool(name="py_f", bufs=2, space="PSUM") as pyf,
        ):
            # token activations: split the bf16 x DMA over two queues so the
            # first phase-1 group can start ASAP.
            xf_sb = cpool.tile([P, KD, KF8], FP8)
            nc.scalar.dma_start(xf_sb[:, 0:KD // 2, :], xf[:, 0:KD // 2, :])
            nc.gpsimd.dma_start(xf_sb[:, KD // 2:KD, :], xf[:, KD // 2:KD, :])
            w1f_t0 = w1fpool.tile([P, 2, KD // 2, 2, P], FP8, tag="w1f")
            nc.sync.dma_start(w1f_t0[:], w1f[0])
            xb_sb = cpool.tile([P, KD, KB], BF16)
            nc.sync.dma_start(xb_sb[:, 0:KD // 2, :], xb[:, 0:KD // 2, :])
            nc.scalar.dma_start(xb_sb[:, KD // 2:KD, :], xb[:, KD // 2:KD, :])
            grep_sb = cpool.tile([P, C], F32)

            hb_sb = cpool.tile([P, KF, KB], BF16)
            hf_sb = cpool.tile([P, KF, KF8], FP8)

            # phase 1: h = gelu(x @ w1); bf16 tile then fp8 tile per
            # f-chunk; w1 streams in two-f-chunk transfers to halve the
            # completion-event count.
            for f in range(KF):
                if f % 2 == 0:
                    w1b_t = w1bpool.tile([P, 2, KD, P], BF16, tag="w1b")
                    nc.gpsimd.dma_start(w1b_t[:], w1b[f // 2])
                    if f == 0:
                        w1f_t = w1f_t0
                    else:
                        w1f_t = w1fpool.tile([P, 2, KD // 2, 2, P], FP8,
                                             tag="w1f")
                        nc.sync.dma_start(w1f_t[:], w1f[f // 2])
                if f == 6:
                    nc.scalar.dma_start(grep_sb[:], grep[:])
                fi = f % 2
                for t in range(NF):
                    ps = phf.tile([P, 512], F32)
                    for kk in range(KD // 2):
                        nc.tensor.matmul(
                            ps[:],
                            lhsT=w1f_t[:, fi, kk, :, :],
                            rhs=xf_sb[:, 2 * kk:2 * kk + 2, ts(t, 512)],
                            start=(kk == 0),
                            stop=(kk == KD // 2 - 1),
                            perf_mode=DR,
                        )
                    nc.scalar.activation(
                        hf_sb[:, f, ts(t, 512)], ps[:],
                        mybir.ActivationFunctionType.Gelu,
                        scale=1.0 / (SX * SW),
                    )
                for t in range(NB):
                    ps = phb.tile([P, 512], F32)
                    if f == 0:
                        # ramp the PE clock while the larger xb/w1b inputs
                        # stream in: back-to-back dummy matmuls on the
                        # already-resident fp8 tiles keep the PE busy through
                        # the DMA-bound warmup window. They write this psum
                        # tile, which the real k==0 matmul resets (start=True).
                        for _ in range(10):
                            nc.tensor.matmul(
                                ps[:], lhsT=w1f_t0[:, 0, 0, 0, :],
                                rhs=xf_sb[:, 0, :],
                                start=True, stop=True,
                            )
                    for k in range(KD):
                        nc.tensor.matmul(
                            ps[:],
                            lhsT=w1b_t[:, fi, k, :],
                            rhs=xb_sb[:, k, ts(t, 512)],
                            start=(k == 0),
                            stop=(k == KD - 1),
                        )
                    nc.scalar.activation(
                        hb_sb[:, f, ts(t, 512)], ps[:],
                        mybir.ActivationFunctionType.Gelu,
                    )

            # phase 2: yT[d, c] = gate[c] * sum_f w2[f, d] * h[f, c]
            for d in range(KD):
                w2b_t = w2bpool.tile([P, KF, P], BF16, tag="w2b")
                nc.gpsimd.dma_start(w2b_t[:], w2b[d])
                w2f_t = w2fpool.tile([P, KF // 2, 2, P], FP8, tag="w2f")
                nc.gpsimd.dma_start(w2f_t[:], w2f[d])
                for t in range(NB):
                    ps = pyb.tile([P, 512], F32)
                    for k in range(KF):
                        nc.tensor.matmul(
                            ps[:],
                            lhsT=w2b_t[:, k, :],
                            rhs=hb_sb[:, k, ts(t, 512)],
                            start=(k == 0),
                            stop=(k == KF - 1),
                        )
                    y_sb = ypool.tile([P, 512], F32, tag="yb")
                    nc.vector.tensor_mul(
                        out=y_sb[:], in0=ps[:], in1=grep_sb[:, ts(t, 512)]
                    )
                    nc.sync.dma_start(yT[ts(d, P), ts(t, 512)], y_sb[:])
                for t in range(NF):
                    ps = pyf.tile([P, 512], F32)
                    for kk in range(KF // 2):
                        nc.tensor.matmul(
                            ps[:],
                            lhsT=w2f_t[:, kk, :, :],
                            rhs=hf_sb[:, 2 * kk:2 * kk + 2, ts(t, 512)],
                            start=(kk == 0),
                            stop=(kk == KF // 2 - 1),
                            perf_mode=DR,
                        )
                    y_sb = ypool.tile([P, 512], F32, tag="yf")
                    nc.vector.tensor_mul(
                        out=y_sb[:], in0=ps[:],
                        in1=grep_sb[:, KB + t * 512:KB + (t + 1) * 512],
                    )
                    nc.sync.dma_start(
                        yT[ts(d, P), KB + t * 512:KB + (t + 1) * 512], y_sb[:]
                    )
    return _legalize_waits(nc)


_BUILT = {}


def _get_router():
    if "router" not in _BUILT:
        _BUILT["router"] = build_router()
    return _BUILT["router"]


def _get_mlp(KB, KF8):
    key = ("mlp", KB, KF8)
    if key not in _BUILT:
        _BUILT[key] = build_mlp_mixed(KB, KF8)
    return _BUILT[key]


def _run(name, nc, in_maps):
    kw = {}
    if PROFILE:
        kw["trace"] = True
    res = run_bass_kernel_spmd(nc, in_maps, core_ids=list(range(N_CORES)), **kw)
    if PROFILE:
        LAST_EXEC_NS[name] = res.exec_time_ns
        LAST_TRACE_DIRS[name] = getattr(res, "profile_json", None)
    return res.results


# ---------------------------------------------------------------------------
# host-side packing helpers
# ---------------------------------------------------------------------------
def _part3(a, np_dt):
    """[K*P, N] -> [P, K, N] with partition dim first (contiguous)."""
    kp, n = a.shape
    k = kp // P
    return np.ascontiguousarray(
        a.reshape(k, P, n).transpose(1, 0, 2).astype(np_dt, copy=False)
    )


def _xT_pack(xg, np_dt):
    """[C, D] tokens -> [P, KD, C] (d-major, partition-first)."""
    return _part3(np.ascontiguousarray(xg.T), np_dt)


def kernel(x, router_w, router_b, w1, w2):
    x = np.asarray(x, dtype=np.float32)
    router_w = np.asarray(router_w, dtype=np.float32)
    router_b = np.asarray(router_b, dtype=np.float32)
    w1 = np.asarray(w1, dtype=np.float32)
    w2 = np.asarray(w2, dtype=np.float32)

    xf_ = x.reshape(NTOK, D)

    # ---- launch 1: router -------------------------------------------------
    rw_f = _part3(router_w, np.float32)                    # [128, 8, 9]
    rw_hi = rw_f.astype(NP_BF16)
    rw_lo = (rw_f - rw_hi.astype(np.float32)).astype(NP_BF16)
    rw2_h = np.ascontiguousarray(
        np.stack([rw_hi, rw_lo], axis=2))                  # [128, 8, 2, 9]
    rbT_h = np.ascontiguousarray(router_b.reshape(E + 1, 1))
    id9_h = np.eye(E + 1, dtype=np.float32)
    in_maps = []
    for c in range(N_CORES):
        xs = xf_[c * TPC:(c + 1) * TPC]                    # [512, 1024]
        xT_h = np.ascontiguousarray(xs.T).reshape(KD, P, TPC)  # [8, 128, 512]
        xh_h = xT_h.astype(NP_BF16)
        xl_h = (xT_h - xh_h.astype(np.float32)).astype(NP_BF16)
        in_maps.append({"xh": xh_h, "xl": xl_h, "rw2": rw2_h,
                        "rbT": rbT_h, "id9": id9_h})
    results = _run("router", _get_router(), in_maps)
    gates = np.concatenate(
        [np.asarray(r["gates"], dtype=np.float32).reshape(TPC, E + 1)
         for r in results], axis=0
    )                                                      # [4096, 9]

    # ---- host all-to-all dispatch + precision split ----------------------
    idx = [np.nonzero(gates[:, e] > 0)[0] for e in range(E)]
    KB = 512
    # per-expert: the KB highest-gate assignments run in bf16; the rest fp8.
    b_idx, f_idx = [], []
    for e in range(E):
        ide = idx[e]
        r = np.argsort(-gates[ide, e], kind="stable")
        b_idx.append(ide[r[:KB]])
        f_idx.append(ide[r[KB:]])
    max_f = max((len(i) for i in f_idx), default=0)
    KF8 = max(512, ((max_f + 511) // 512) * 512)

    nc_mlp = _get_mlp(KB, KF8)
    in_maps = []
    for e in range(E):
        w1_bl = np.stack(
            [
                w1[e][:, f * P:(f + 1) * P].reshape(KD, P, P).transpose(1, 0, 2)
                for f in range(KF)
            ]
        )                                                  # [32, 128, 8, 128] f32
        w2_bl = np.stack(
            [
                w2[e][:, d * P:(d + 1) * P].reshape(KF, P, P).transpose(1, 0, 2)
                for d in range(KD)
            ]
        )                                                  # [8, 128, 32, 128] f32
        w1b_h = w1_bl.astype(NP_BF16).reshape(KF // 2, 2, P, KD, P) \
            .transpose(0, 2, 1, 3, 4)
        w2b_h = w2_bl.astype(NP_BF16)
        w1f_h = (w1_bl * SW).astype(NP_FP8) \
            .reshape(KF // 2, 2, P, KD // 2, 2, P).transpose(0, 2, 1, 3, 4, 5)
        w2f_h = (w2_bl * SW).astype(NP_FP8).reshape(KD, P, KF // 2, 2, P)

        ib, if8 = b_idx[e], f_idx[e]
        nb, nf = len(ib), len(if8)
        xg_b = np.zeros((KB, D), dtype=np.float32)
        xg_b[:nb] = xf_[ib]
        xg_f = np.zeros((KF8, D), dtype=np.float32)
        xg_f[:nf] = xf_[if8]
        g = np.zeros((KB + KF8,), dtype=np.float32)
        g[:nb] = gates[ib, e]
        g[KB:KB + nf] = gates[if8, e] / (SW)

        in_maps.append({
            "w1b": np.ascontiguousarray(w1b_h),
            "w1f": np.ascontiguousarray(w1f_h),
            "w2b": np.ascontiguousarray(w2b_h),
            "w2f": np.ascontiguousarray(w2f_h),
            "xb": _xT_pack(xg_b, NP_BF16),
            "xf": _xT_pack(xg_f * SX, NP_FP8),
            "grep": np.ascontiguousarray(np.broadcast_to(g, (P, KB + KF8))),
        })

    # ---- launch 2: expert MLP --------------------------------------------
    results = _run("mlp", nc_mlp, in_maps)

    # ---- host combine -----------------------------------------------------
    out = gates[:, E:E + 1] * xf_                          # dummy identity expert
    for e in range(E):
        yT = np.asarray(results[e]["yT"], dtype=np.float32)    # [1024, KB+KF8]
        nb, nf = len(b_idx[e]), len(f_idx[e])
        if nb:
            out[b_idx[e]] += yT[:, :nb].T
        if nf:
            out[f_idx[e]] += yT[:, KB:KB + nf].T
    return out.reshape(B, T, D).astype(np.float32)


# revision 25
# speedup vs baseline: 1.1877x; 1.0034x over previous
"""MoE (top-2 of 8 experts + dummy identity expert) on 8 NeuronCores.

Strategy (expert parallelism, per the sharding hint):
  Launch 1 (router, token-parallel): each core computes logits -> softmax
    -> top-2 gates for its 512-token shard, fully on device (fp32 so the
    top-2 selection matches the fp32 reference bit-for-bit).
  Host all-to-all "dispatch": compact token indices per expert from the
    device-computed gates, gather+transpose token activations.
  Launch 2 (expert MLP, expert-parallel, MIXED PRECISION): core e holds
    expert e's weights. Tokens are split by routing risk s=||gates||_2:
    the top KB tokens per expert run in bf16; the low-gate remainder runs
    in fp8(e4m3) with DoubleRow perf mode (2x PE throughput). Quantization
    error scales with the gate, so low-gate tokens tolerate fp8.
  Host "combine": scatter-add per-expert outputs + dummy-expert term.
"""

import math
import os
import sys

for _p in ("/opt/trn_rl_repo",):
    if _p not in sys.path:
        sys.path.insert(0, _p)

import numpy as np
import ml_dtypes

import concourse.bass as bass
import concourse.mybir as mybir
import concourse.tile as tile
from concourse.bass import ts
from concourse.bass_utils import run_bass_kernel_spmd

# ---------------------------------------------------------------------------
# This container's walrus accepts at most ONE sync-wait command per
# instruction. Tile can attach several (body instructions and the
# kernel-tail drain). Hoist excess waits onto same-engine NoOps inserted
# immediately before the offending instruction — semantically identical
# (waits are AND conditions evaluated in stream order).
# ---------------------------------------------------------------------------
_WAITS_PER_INST = 1
_legalize_counter = [0]


def _legalize_waits(nc):
    for f in nc.m.functions:
        for bb in f.blocks:
            insts = list(bb.instructions)
            out = []
            changed = False
            for inst in insts:
                si = inst.sync_info
                waits = list(si.on_wait) if si is not None and si.on_wait else []
                if len(waits) > _WAITS_PER_INST:
                    changed = True
                    for w in waits[:-_WAITS_PER_INST]:
                        _legalize_counter[0] += 1
                        out.append(
                            mybir.InstNoOp(
                                name=f"legwait-{_legalize_counter[0]}",
                                ins=[],
                                outs=[],
                                engine=inst.engine,
                                sync_info=mybir.SyncInfo(
                                    on_wait=[w], on_update=[]
                                ),
                            )
                        )
                    si.on_wait = waits[-_WAITS_PER_INST:]
                out.append(inst)
            if changed:
                bb.instructions = out
    return nc


# ---------------------------------------------------------------------------
# Problem constants (hardcoded per contract; inputs are fixed-shape).
# ---------------------------------------------------------------------------
N_CORES = 8
B, T, D, F, E = 2, 2048, 1024, 4096, 8
NTOK = B * T            # 4096 tokens
TPC = NTOK // N_CORES   # 512 tokens/core in the router launch
P = 128
KD = D // P             # 8 contraction chunks over D
KF = F // P             # 32 contraction chunks over F

F32 = mybir.dt.float32
BF16 = mybir.dt.bfloat16
FP8 = mybir.dt.float8e4
NP_BF16 = ml_dtypes.bfloat16
NP_FP8 = ml_dtypes.float8_e4m3

# fp8 scaling: pre-psum = (SX*x) @ (SW*w1) = SX*SW*pre; gelu scale undoes it.
SX = 8.0
SW = 64.0

PROFILE = False          # set True (from test.py) to collect NTFF exec times
LAST_EXEC_NS = {}        # launch name -> exec_time_ns (filled when PROFILE)
LAST_TRACE_DIRS = {}


# ---------------------------------------------------------------------------
# Launch 1: router. Per core: 512 tokens -> gates [512, 9].
# ---------------------------------------------------------------------------
def build_router():
    NE = E + 1
    TH = TPC // 2   # 256-token half: two independent pipelines so the
    TTH = TH // P   # first half's softmax chain overlaps the second
    nc = bass.Bass()
    # x and router weights arrive as bf16 hi/lo pairs; computing
    #   x_hi@rw_hi + x_lo@rw_hi + x_hi@rw_lo
    # in fp32 PSUM reproduces fp32 logits to ~1e-5 (verified: every token's
    # error is <= 10% of its top-2 decision gap, so selection is exact)
    # while running the PE at 1 cycle/row instead of fp32's 4. Each token's
    # accumulation order is unchanged by the half-split, so gates are
    # bit-identical to the unsplit version.
    xh = nc.declare_dram_parameter("xh", [KD, P, TPC], BF16, isOutput=False)
    xl = nc.declare_dram_parameter("xl", [KD, P, TPC], BF16, isOutput=False)
    rw2 = nc.declare_dram_parameter("rw2", [P, KD, 2, NE], BF16, isOutput=False)
    rbT = nc.declare_dram_parameter("rbT", [NE, 1], F32, isOutput=False)
    id9 = nc.declare_dram_parameter("id9", [NE, NE], F32, isOutput=False)
    gates = nc.declare_dram_parameter("gates", [2, TH, NE], F32, isOutput=True)

    with tile.TileContext(nc) as tc:
        with (
            tc.tile_pool(name="const", bufs=1) as cpool,
            tc.tile_pool(name="xp", bufs=8) as xpool,
            tc.tile_pool(name="work", bufs=2) as pool,
            tc.tile_pool(name="psum", bufs=1, space="PSUM") as pp,
            tc.tile_pool(name="psum2", bufs=1, space="PSUM") as pp2,
        ):
            rw_sb = cpool.tile([P, KD, 2, NE], BF16)
            nc.sync.dma_start(rw_sb[:], rw2[:])
            rbT_sb = cpool.tile([NE, 1], F32)
            id9_sb = cpool.tile([NE, NE], F32)

            dma_engines = [nc.sync, nc.gpsimd, nc.scalar]
            qi = [0]

            def nxt():
                e = dma_engines[qi[0] % 3]
                qi[0] += 1
                return e

            for half in range(2):
                tok = ts(half, TH)
                ps_lgT = pp2.tile([NE, TH], F32, name=f"pslg{half}")
                for k in range(KD):
                    xh_sb = xpool.tile([P, TH], BF16, tag=f"xh{half}")
                    nxt().dma_start(xh_sb[:], xh[k][:, tok])
                    xl_sb = xpool.tile([P, TH], BF16, tag=f"xl{half}")
                    nxt().dma_start(xl_sb[:], xl[k][:, tok])
                    if half == 0 and k == 1:
                        nc.scalar.dma_start(rbT_sb[:], rbT[:])
                        nc.scalar.dma_start(id9_sb[:], id9[:])
                    if half == 0 and k == 0:
                        # ramp the PE clock during the DMA-bound head:
                        # dummy matmuls on the first-arriving chunk, written
                        # into this psum tile, which the real k==0 matmul
                        # resets (start=True).
                        for _ in range(12):
                            nc.tensor.matmul(
                                ps_lgT[:, 0:NE], lhsT=rw_sb[:, 0, 0, :],
                                rhs=xh_sb[:, 0:NE],
                                start=True, stop=True,
                            )
                        for _ in range(12):
                            nc.tensor.matmul(
                                ps_lgT[:], lhsT=rw_sb[:, 0, 0, :],
                                rhs=xh_sb[:],
                                start=True, stop=True,
                            )
                    nc.tensor.matmul(
                        ps_lgT[:], lhsT=rw_sb[:, k, 0, :], rhs=xh_sb[:],
                        start=(k == 0), stop=False,
                    )
                    nc.tensor.matmul(
                        ps_lgT[:], lhsT=rw_sb[:, k, 0, :], rhs=xl_sb[:],
                        start=False, stop=False,
                    )
                    nc.tensor.matmul(
                        ps_lgT[:], lhsT=rw_sb[:, k, 1, :], rhs=xh_sb[:],
                        start=False, stop=(k == KD - 1),
                    )
                lgT_sb = pool.tile([NE, TH], F32, tag=f"lgT{half}")
                nc.vector.tensor_tensor(
                    lgT_sb[:], ps_lgT[:],
                    rbT_sb[:].to_broadcast([NE, TH]),
                    mybir.AluOpType.add,
                )
                # transpose back to token-major [128, TTH, 9] via PE
                ps = pp.tile([P, TTH, NE], F32, name=f"pst{half}")
                for tt in range(TTH):
                    nc.tensor.transpose(
                        ps[:, tt, :], lgT_sb[:, ts(tt, P)], id9_sb[:]
                    )

                # logits are O(+-5): exp() cannot overflow fp32, and the
                # softmax normalization cancels any shift, so skip the max-
                # subtraction entirely (selection is order-preserving).
                sh3 = [P, TTH, NE]
                e_sb = pool.tile(sh3, F32, tag=f"e{half}")
                nc.scalar.activation(
                    e_sb[:], ps[:], mybir.ActivationFunctionType.Exp,
                )
                s = pool.tile([P, TTH], F32, tag=f"s{half}")
                nc.vector.tensor_reduce(
                    s[:], e_sb[:], axis=mybir.AxisListType.X,
                    op=mybir.AluOpType.add,
                )
                mx = pool.tile([P, TTH], F32, tag=f"mx{half}")
                nc.vector.tensor_reduce(
                    mx[:], e_sb[:], axis=mybir.AxisListType.X,
                    op=mybir.AluOpType.max,
                )
                # knock out the top-1, take max again -> second-largest
                mlt = pool.tile(sh3, F32, tag=f"mlt{half}")
                nc.vector.tensor_tensor(
                    mlt[:], e_sb[:], mx[:, :, None].to_broadcast(sh3),
                    mybir.AluOpType.is_lt,
                )
                emask = pool.tile(sh3, F32, tag=f"emask{half}")
                nc.vector.tensor_mul(out=emask[:], in0=mlt[:], in1=e_sb[:])
                m2 = pool.tile([P, TTH], F32, tag=f"m2{half}")
                nc.vector.tensor_reduce(
                    m2[:], emask[:], axis=mybir.AxisListType.X,
                    op=mybir.AluOpType.max,
                )
                gmask = pool.tile(sh3, F32, tag=f"gmask{half}")
                nc.vector.tensor_tensor(
                    gmask[:], e_sb[:], m2[:, :, None].to_broadcast(sh3),
                    mybir.AluOpType.is_ge,
                )
                gsel = pool.tile(sh3, F32, tag=f"gsel{half}")
                nc.vector.tensor_mul(out=gsel[:], in0=gmask[:], in1=e_sb[:])
                rs = pool.tile([P, TTH], F32, tag=f"rs{half}")
                nc.vector.reciprocal(rs[:], s[:])
                gfin = pool.tile(sh3, F32, tag=f"gfin{half}")
                nc.vector.tensor_tensor(
                    gfin[:], gsel[:], rs[:, :, None].to_broadcast(sh3),
                    mybir.AluOpType.mult,
                )
                nc.sync.dma_start(
                    gates[half].rearrange("(tt p) e -> p tt e", p=P), gfin[:]
                )
    return _legalize_waits(nc)


# ---------------------------------------------------------------------------
# Launch 2: mixed-precision expert MLP. Per core: KB bf16 tokens + KF8 fp8
# tokens through one expert.
#   yT[d, c] = gate[c] * (gelu(x @ w1) @ w2)[c, d]   (transposed output)
# fp8 tiles use DoubleRow perf mode: lhsT [128,2,128] x rhs [128,2,512]
# contracts 256 elements per instruction at 2x bf16 throughput.
# ---------------------------------------------------------------------------
def build_mlp_mixed(KB, KF8):
    assert KB % 512 == 0 and KF8 % 512 == 0
    NB = KB // 512
    NF = KF8 // 512
    C = KB + KF8
    DR = mybir.MatmulPerfMode.DoubleRow
    nc = bass.Bass()
    w1b = nc.declare_dram_parameter("w1b", [KF // 2, P, 2, KD, P], BF16,
                                    isOutput=False)
    w1f = nc.declare_dram_parameter("w1f", [KF // 2, P, 2, KD // 2, 2, P], FP8,
                                    isOutput=False)
    w2b = nc.declare_dram_parameter("w2b", [KD, P, KF, P], BF16, isOutput=False)
    w2f = nc.declare_dram_parameter("w2f", [KD, P, KF // 2, 2, P], FP8,
                                    isOutput=False)
    xb = nc.declare_dram_parameter("xb", [P, KD, KB], BF16, isOutput=False)
    xf = nc.declare_dram_parameter("xf", [P, KD, KF8], FP8, isOutput=False)
    grep = nc.declare_dram_parameter("grep", [P, C], F32, isOutput=False)
    yT = nc.declare_dram_parameter("yT", [D, C], F32, isOutput=True)

    with tile.TileContext(nc) as tc:
        with (
            tc.tile_pool(name="const", bufs=1) as cpool,
            tc.tile_pool(name="w1bp", bufs=3) as w1bpool,
            tc.tile_pool(name="w1fp", bufs=3) as w1fpool,
            tc.tile_pool(name="w2bp", bufs=2) as w2bpool,
            tc.tile_pool(name="w2fp", bufs=2) as w2fpool,
            tc.tile_pool(name="yp", bufs=4) as ypool,
            tc.tile_pool(name="ph_b", bufs=2, space="PSUM") as phb,
            tc.tile_pool(name="ph_f", bufs=2, space="PSUM") as phf,
            tc.tile_pool(name="py_b", bufs=2, space="PSUM") as pyb,
            tc.tile_pool(name="py_f", bufs=2, space="PSUM") as pyf,
        ):
            # token activations: split the bf16 x DMA over two queues so the
            # first phase-1 group can start ASAP.
            xf_sb = cpool.tile([P, KD, KF8], FP8)
            nc.scalar.dma_start(xf_sb[:, 0:KD // 2, :], xf[:, 0:KD // 2, :])
            nc.gpsimd.dma_start(xf_sb[:, KD // 2:KD, :], xf[:, KD // 2:KD, :])
            w1f_t0 = w1fpool.tile([P, 2, KD // 2, 2, P], FP8, tag="w1f")
            nc.sync.dma_start(w1f_t0[:], w1f[0])
            xb_sb = cpool.tile([P, KD, KB], BF16)
            nc.sync.dma_start(xb_sb[:, 0:KD // 2, :], xb[:, 0:KD // 2, :])
            nc.scalar.dma_start(xb_sb[:, KD // 2:KD, :], xb[:, KD // 2:KD, :])
            grep_sb = cpool.tile([P, C], F32)

            hb_sb = cpool.tile([P, KF, KB], BF16)
            hf_sb = cpool.tile([P, KF, KF8], FP8)

            # phase 1: h = gelu(x @ w1); bf16 tile then fp8 tile per
            # f-chunk; w1 streams in two-f-chunk transfers to halve the
            # completion-event count.
            for f in range(KF):
                if f % 2 == 0:
                    w1b_t = w1bpool.tile([P, 2, KD, P], BF16, tag="w1b")
                    nc.gpsimd.dma_start(w1b_t[:], w1b[f // 2])
                    if f == 0:
                        w1f_t = w1f_t0
                    else:
                        w1f_t = w1fpool.tile([P, 2, KD // 2, 2, P], FP8,
                                             tag="w1f")
                        nc.sync.dma_start(w1f_t[:], w1f[f // 2])
                if f == 6:
                    nc.scalar.dma_start(grep_sb[:], grep[:])
                fi = f % 2
                for t in range(NF):
                    ps = phf.tile([P, 512], F32)
                    for kk in range(KD // 2):
                        nc.tensor.matmul(
                            ps[:],
                            lhsT=w1f_t[:, fi, kk, :, :],
                            rhs=xf_sb[:, 2 * kk:2 * kk + 2, ts(t, 512)],
                            start=(kk == 0),
                            stop=(kk == KD // 2 - 1),
                            perf_mode=DR,
                        )
                    nc.scalar.activation(
                        hf_sb[:, f, ts(t, 512)], ps[:],
                        mybir.ActivationFunctionType.Gelu,
                        scale=1.0 / (SX * SW),
                    )
                for t in range(NB):
                    ps = phb.tile([P, 512], F32)
                    if f == 0:
                        # ramp the PE clock while the larger xb/w1b inputs
                        # stream in: back-to-back dummy matmuls on the
                        # already-resident fp8 tiles keep the PE busy through
                        # the DMA-bound warmup window. They write this psum
                        # tile, which the real k==0 matmul resets (start=True).
                        for _ in range(10):
                            nc.tensor.matmul(
                                ps[:], lhsT=w1f_t0[:, 0, 0, 0, :],
                                rhs=xf_sb[:, 0, :],
                                start=True, stop=True,
                            )
                    for k in range(KD):
                        nc.tensor.matmul(
                            ps[:],
                            lhsT=w1b_t[:, fi, k, :],
                            rhs=xb_sb[:, k, ts(t, 512)],
                            start=(k == 0),
                            stop=(k == KD - 1),
                        )
                    nc.scalar.activation(
                        hb_sb[:, f, ts(t, 512)], ps[:],
                        mybir.ActivationFunctionType.Gelu,
                    )

            # phase 2: yT[d, c] = gate[c] * sum_f w2[f, d] * h[f, c]
            for d in range(KD):
                w2b_t = w2bpool.tile([P, KF, P], BF16, tag="w2b")
                nc.gpsimd.dma_start(w2b_t[:], w2b[d])
                w2f_t = w2fpool.tile([P, KF // 2, 2, P], FP8, tag="w2f")
                nc.gpsimd.dma_start(w2f_t[:], w2f[d])
                for t in range(NB):
                    ps = pyb.tile([P, 512], F32)
                    for k in range(KF):
                        nc.tensor.matmul(
                            ps[:],
                            lhsT=w2b_t[:, k, :],
                            rhs=hb_sb[:, k, ts(t, 512)],
                            start=(k == 0),
                            stop=(k == KF - 1),
                        )
                    y_sb = ypool.tile([P, 512], F32, tag="yb")
                    nc.vector.tensor_mul(
                        out=y_sb[:], in0=ps[:], in1=grep_sb[:, ts(t, 512)]
                    )
                    nc.sync.dma_start(yT[ts(d, P), ts(t, 512)], y_sb[:])
                for t in range(NF):
                    ps = pyf.tile([P, 512], F32)
                    for kk in range(KF // 2):
                        nc.tensor.matmul(
                            ps[:],
                            lhsT=w2f_t[:, kk, :, :],
                            rhs=hf_sb[:, 2 * kk:2 * kk + 2, ts(t, 512)],
                            start=(kk == 0),
                            stop=(kk == KF // 2 - 1),
                            perf_mode=DR,
                        )
                    y_sb = ypool.tile([P, 512], F32, tag="yf")
                    nc.vector.tensor_mul(
                        out=y_sb[:], in0=ps[:],
                        in1=grep_sb[:, KB + t * 512:KB + (t + 1) * 512],
                    )
                    nc.sync.dma_start(
                        yT[ts(d, P), KB + t * 512:KB + (t + 1) * 512], y_sb[:]
                    )
    return _legalize_waits(nc)


_BUILT = {}


def _get_router():
    if "router" not in _BUILT:
        _BUILT["router"] = build_router()
    return _BUILT["router"]


def _get_mlp(KB, KF8):
    key = ("mlp", KB, KF8)
    if key not in _BUILT:
        _BUILT[key] = build_mlp_mixed(KB, KF8)
    return _BUILT[key]


def _run(name, nc, in_maps):
    kw = {}
    if PROFILE:
        kw["trace"] = True
    res = run_bass_kernel_spmd(nc, in_maps, core_ids=list(range(N_CORES)), **kw)
    if PROFILE:
        LAST_EXEC_NS[name] = res.exec_time_ns
        LAST_TRACE_DIRS[name] = getattr(res, "profile_json", None)
    return res.results


# ---------------------------------------------------------------------------
# host-side packing helpers
# ---------------------------------------------------------------------------
def _part3(a, np_dt):
    """[K*P, N] -> [P, K, N] with partition dim first (contiguous)."""
    kp, n = a.shape
    k = kp // P
    return np.ascontiguousarray(
        a.reshape(k, P, n).transpose(1, 0, 2).astype(np_dt, copy=False)
    )


def _xT_pack(xg, np_dt):
    """[C, D] tokens -> [P, KD, C] (d-major, partition-first)."""
    return _part3(np.ascontiguousarray(xg.T), np_dt)


def kernel(x, router_w, router_b, w1, w2):
    x = np.asarray(x, dtype=np.float32)
    router_w = np.asarray(router_w, dtype=np.float32)
    router_b = np.asarray(router_b, dtype=np.float32)
    w1 = np.asarray(w1, dtype=np.float32)
    w2 = np.asarray(w2, dtype=np.float32)

    xf_ = x.reshape(NTOK, D)

    # ---- launch 1: router -------------------------------------------------
    rw_f = _part3(router_w, np.float32)                    # [128, 8, 9]
    rw_hi = rw_f.astype(NP_BF16)
    rw_lo = (rw_f - rw_hi.astype(np.float32)).astype(NP_BF16)
    rw2_h = np.ascontiguousarray(
        np.stack([rw_hi, rw_lo], axis=2))                  # [128, 8, 2, 9]
    rbT_h = np.ascontiguousarray(router_b.reshape(E + 1, 1))
    id9_h = np.eye(E + 1, dtype=np.float32)
    in_maps = []
    for c in range(N_CORES):
        xs = xf_[c * TPC:(c + 1) * TPC]                    # [512, 1024]
        xT_h = np.ascontiguousarray(xs.T).reshape(KD, P, TPC)  # [8, 128, 512]
        xh_h = xT_h.astype(NP_BF16)
        xl_h = (xT_h - xh_h.astype(np.float32)).astype(NP_BF16)
        in_maps.append({"xh": xh_h, "xl": xl_h, "rw2": rw2_h,
                        "rbT": rbT_h, "id9": id9_h})
    results = _run("router", _get_router(), in_maps)
    gates = np.concatenate(
        [np.asarray(r["gates"], dtype=np.float32).reshape(TPC, E + 1)
         for r in results], axis=0
    )                                                      # [4096, 9]

    # ---- host all-to-all dispatch + precision split ----------------------
    idx = [np.nonzero(gates[:, e] > 0)[0] for e in range(E)]
    KB = 512
    # per-expert: the KB highest-gate assignments run in bf16; the rest fp8.
    b_idx, f_idx = [], []
    for e in range(E):
        ide = idx[e]
        r = np.argsort(-gates[ide, e], kind="stable")
        b_idx.append(ide[r[:KB]])
        f_idx.append(ide[r[KB:]])
    max_f = max((len(i) for i in f_idx), default=0)
    KF8 = max(512, ((max_f + 511) // 512) * 512)

    nc_mlp = _get_mlp(KB, KF8)
    in_maps = []
    for e in range(E):
        w1_bl = np.stack(
            [
                w1[e][:, f * P:(f + 1) * P].reshape(KD, P, P).transpose(1, 0, 2)
                for f in range(KF)
            ]
        )                                                  # [32, 128, 8, 128] f32
        w2_bl = np.stack(
            [
                w2[e][:, d * P:(d + 1) * P].reshape(KF, P, P).transpose(1, 0, 2)
                for d in range(KD)
            ]
        )                                                  # [8, 128, 32, 128] f32
        w1b_h = w1_bl.astype(NP_BF16).reshape(KF // 2, 2, P, KD, P) \
            .transpose(0, 2, 1, 3, 4)
        w2b_h = w2_bl.astype(NP_BF16)
        w1f_h = (w1_bl * SW).astype(NP_FP8) \
            .reshape(KF // 2, 2, P, KD // 2, 2, P).transpose(0, 2, 1, 3, 4, 5)
        w2f_h = (w2_bl * SW).astype(NP_FP8).reshape(KD, P, KF // 2, 2, P)

        ib, if8 = b_idx[e], f_idx[e]
        nb, nf = len(ib), len(if8)
        xg_b = np.zeros((KB, D), dtype=np.float32)
        xg_b[:nb] = xf_[ib]
        xg_f = np.zeros((KF8, D), dtype=np.float32)
        xg_f[:nf] = xf_[if8]
        g = np.zeros((KB + KF8,), dtype=np.float32)
        g[:nb] = gates[ib, e]
        g[KB:KB + nf] = gates[if8, e] / (SW)

        in_maps.append({
            "w1b": np.ascontiguousarray(w1b_h),
            "w1f": np.ascontiguousarray(w1f_h),
            "w2b": np.ascontiguousarray(w2b_h),
            "w2f": np.ascontiguousarray(w2f_h),
            "xb": _xT_pack(xg_b, NP_BF16),
            "xf": _xT_pack(xg_f * SX, NP_FP8),
            "grep": np.ascontiguousarray(np.broadcast_to(g, (P, KB + KF8))),
        })

    # ---- launch 2: expert MLP --------------------------------------------
    results = _run("mlp", nc_mlp, in_maps)

    # ---- host combine -----------------------------------------------------
    out = gates[:, E:E + 1] * xf_                          # dummy identity expert
    for e in range(E):
        yT = np.asarray(results[e]["yT"], dtype=np.float32)    # [1024, KB+KF8]
        nb, nf = len(b_idx[e]), len(f_idx[e])
        if nb:
            out[b_idx[e]] += yT[:, :nb].T
        if nf:
            out[f_idx[e]] += yT[:, KB:KB + nf].T
    return out.reshape(B, T, D).astype(np.float32)
